# revision 24
# baseline (speedup 1.0000x reference)
"""Trainium2 Bass kernel for a dense transformer block (B=4,S=1024,D=1024,F=4096,H=16).

Sharding: 8 cores = (batch b in 0..3) x (seq half). Pure SPMD, no collectives:
the host rotates each core's tokens so its 512 query rows are always rows
0..511 of the rotated sequence; K/V cover the full (rotated) sequence.

v2: fp8e4m3 DoubleRow matmuls (0.5 cyc/row) for the QKV projections and Wo.
  - y1 = LN1(x) transposed to a single interleaved fp8 tile y1T8 [128, 8, 1024]
    so each DoubleRow matmul contracts 256 features (2 chunks of 128).
  - weights are pre-scaled on host (Wq*128, Wk*16, Wv*16, Wo*16) to lift the
    0.02-scale entries out of fp8 subnormal range; the score exp absorbs
    1/(128*16) via its activation scale, the softmax ones-column is set to 16
    to absorb V's scale, and an ACT copy with scale=1/16 rescales Wo's output.
  - scores/ctx and the FFN stay bf16 (fp8 there costs too much accuracy).
Scheduling: x DMAs lead the SP queue (weights follow, FFN weights prefetched
through outer-scope stream pools), LN runs split 0-3 / 4-7 so qT and the kT
half-chunks start as soon as their token tiles are normalized, exps are
batched over kt-pairs ([128,1024] PSUM reads), mask-multiply + LN1 apply +
V-ones memsets + part of the copies/adds run on GpSimd(Pool).
The softmax row-sum rides the ctx matmul as 16.0-columns of V_aug; the
per-(b,h) additive attn_bias is constant over q and k so softmax cancels it.
"""

import numpy as np
import ml_dtypes

import concourse.bass as bass
import concourse.mybir as mybir
import concourse.tile as tile
from concourse import bacc
from concourse.bass_utils import run_bass_kernel_spmd

F32 = mybir.dt.float32
BF16 = mybir.dt.bfloat16
F8 = mybir.dt.float8e4
BF = ml_dtypes.bfloat16
E4 = ml_dtypes.float8_e4m3

B, S, D, F, H = 4, 1024, 1024, 4096, 16
d = D // H          # 64
P = 128             # partitions
SQ = 512            # queries per core
EPS = 1e-5
NT = S // P         # 8 token tiles (full seq)
NQ = SQ // P        # 4 query tiles
NF = D // P         # 8 feature chunks
NJ = NF // 2        # 4 fp8 contraction pairs
NF1 = F // P        # 32 ffn chunks

SCQ = 128.0         # host pre-scale on Wq (includes d^-0.5 fold)
SCK = 16.0          # host pre-scale on Wk
SCV = 16.0          # host pre-scale on Wv (absorbed by 16.0 ones-column)
SCO = 16.0          # host pre-scale on Wo (rescaled by ACT copy 1/16)
SC1 = 16.0          # pre-scale on W1 (both halves; gelu scale absorbs 1/16)
SC2 = 16.0          # pre-scale on W2 (both halves; ACT copy absorbs 1/16)
NA1 = 4             # f_in chunks of FFN1 run in fp8 (alpha = NA1/NF)
NA2 = 2             # f1 chunks of FFN2 run in fp8 (alpha = NA2/NF1)

AX = mybir.AxisListType
ALU = mybir.AluOpType
ACTF = mybir.ActivationFunctionType
DR = mybir.MatmulPerfMode.DoubleRow


def _T(pool, shape, dtype, tag):
    return pool.tile(shape, dtype, name=tag, tag=tag)


def _pbcast(ap, p):
    """Partition-broadcast a [1, N] DRAM AP to [p, N]."""
    return bass.AP(tensor=ap.tensor, offset=ap.offset, ap=[[0, p]] + list(ap.ap[1:]))


def _build_program(FL, reps=1):
    nc = bacc.Bacc("TRN2", target_bir_lowering=False, debug=False)

    t = {}
    t["x"] = nc.dram_tensor("x", [SQ, D], F32, kind="ExternalInput").ap()
    t["xk"] = nc.dram_tensor("xk", [SQ, D], BF16, kind="ExternalInput").ap()
    t["maskT2"] = nc.dram_tensor("maskT2", [NT // 2, P, 2 * SQ], mybir.dt.uint8,
                                 kind="ExternalInput").ap()
    t["wq"] = nc.dram_tensor("wq", [NJ, P, 2, D], F8, kind="ExternalInput").ap()
    t["wk"] = nc.dram_tensor("wk", [NJ, P, 2, D], F8, kind="ExternalInput").ap()
    t["wv"] = nc.dram_tensor("wv", [NJ, P, 2, D], F8, kind="ExternalInput").ap()
    t["wo"] = nc.dram_tensor("wo", [2 * NJ, P, 2, SQ], F8, kind="ExternalInput").ap()
    t["w1a"] = nc.dram_tensor("w1a", [NA1 // 2, P, 2, F], F8,
                              kind="ExternalInput").ap()
    t["w1b"] = nc.dram_tensor("w1b", [NF1, P, (NF - NA1) * P], BF16,
                              kind="ExternalInput").ap()
    t["w2a"] = nc.dram_tensor("w2a", [2 * (NA2 // 2), P, 2, SQ], F8,
                              kind="ExternalInput").ap()
    t["w2b"] = nc.dram_tensor("w2b", [2 * (NF1 - NA2), P, SQ], BF16,
                              kind="ExternalInput").ap()
    t["bq"] = nc.dram_tensor("bq", [1, D], BF16, kind="ExternalInput").ap()
    t["bk"] = nc.dram_tensor("bk", [1, D], BF16, kind="ExternalInput").ap()
    t["bv"] = nc.dram_tensor("bv", [1, D], BF16, kind="ExternalInput").ap()
    t["bo"] = nc.dram_tensor("bo", [1, D], F32, kind="ExternalInput").ap()
    t["b1"] = nc.dram_tensor("b1", [1, F], BF16, kind="ExternalInput").ap()
    t["b2"] = nc.dram_tensor("b2", [1, D], BF16, kind="ExternalInput").ap()
    t["ident"] = nc.dram_tensor("ident", [P, P], BF16, kind="ExternalInput").ap()
    t["out"] = nc.dram_tensor("out", [SQ, D], F32, kind="ExternalOutput").ap()

    with tile.TileContext(nc) as tc:
        for rep in range(reps):
            _trace(nc, tc, t, FL, pfx=f"r{rep}_" if reps > 1 else "")
    nc.compile()
    return nc


def _ln_stats(nc, pool, x_ap, epst):
    """Return [P,1] mean and rstd tiles for a [P, 1024] input."""
    st = _T(pool, [P, 2, 6], F32, "st")
    xr = x_ap.rearrange("p (a b) -> p a b", b=512)
    for sg in range(2):
        nc.vector.bn_stats(out=st[:, sg, :], in_=xr[:, sg, :])
    mv = _T(pool, [P, 2], F32, "mv")
    nc.vector.bn_aggr(out=mv[:], in_=st[:])
    rs = _T(pool, [P, 1], F32, "rs")
    nc.scalar.activation(out=rs[:], in_=mv[:, 1:2], func=ACTF.Sqrt,
                         bias=epst[:], scale=1.0)
    nc.vector.reciprocal(out=rs[:], in_=rs[:])
    return mv, rs


def _trace(nc, tc, t, FL, pfx=""):
    x3 = t["x"].rearrange("(n p) c -> n p c", p=P)
    xk3 = t["xk"].rearrange("(n p) c -> n p c", p=P)
    out3 = t["out"].rearrange("(n p) c -> n p c", p=P)

    with (
        tc.tile_pool(name=pfx + "const", bufs=1) as const_p,
        tc.tile_pool(name=pfx + "small", bufs=4) as small_p,
        tc.tile_pool(name=pfx + "xres", bufs=NQ) as xres_p,
        tc.tile_pool(name=pfx + "ctxT", bufs=1) as ctxT_p,
        tc.tile_pool(name=pfx + "w1a", bufs=NA1 // 2) as w1a_p,
        tc.tile_pool(name=pfx + "w1s", bufs=6) as w1_p,
        tc.tile_pool(name=pfx + "w2s", bufs=10) as wh_p,
    ):
        # ---- constants ----
        ident = _T(const_p, [P, P], BF16, "ident")
        nc.gpsimd.dma_start(out=ident[:], in_=t["ident"])
        if any(FL[k] for k in ("bq", "bk", "bv", "b1", "b2")):
            ones = _T(const_p, [1, SQ], BF16, "ones")
            nc.vector.memset(ones[:], 1.0)
        epst = _T(const_p, [P, 1], F32, "epst")
        nc.vector.memset(epst[:], EPS)
        if FL["bo"]:
            bo_b = _T(const_p, [P, D], F32, "bo_b")
            nc.sync.dma_start(out=bo_b[:], in_=_pbcast(t["bo"], P))
        brow = {}
        for nm in ("bq", "bk", "bv", "b1", "b2"):
            if not FL[nm]:
                continue
            sz = F if nm == "b1" else D
            brow[nm] = _T(const_p, [1, sz], BF16, f"brow_{nm}")
            nc.sync.dma_start(out=brow[nm][:], in_=t[nm])

        xres = [_T(xres_p, [P, D], F32, "xres") for _ in range(NQ)]
        ctxT8 = _T(ctxT_p, [P, NF, SQ], F8, "ctxT8")

        # ================= phase 1: LN1, y1T8, QKV, attention =================
        with (
            tc.tile_pool(name=pfx + "y1T8", bufs=1) as y1T8_p,
            tc.tile_pool(name=pfx + "qkT", bufs=1) as qkT_p,
            tc.tile_pool(name=pfx + "wqk", bufs=NJ) as wqk_p,
            tc.tile_pool(name=pfx + "wv", bufs=NJ) as wv_p,
            tc.tile_pool(name=pfx + "Vp", bufs=NT) as V_p,
            tc.tile_pool(name=pfx + "mk", bufs=NT // 2) as mk_p,
            tc.tile_pool(name=pfx + "es", bufs=10) as e_p,
            tc.tile_pool(name=pfx + "xs", bufs=4) as xs_p,
            tc.tile_pool(name=pfx + "y1", bufs=3) as y1_p,
        ):
            y1T8 = _T(y1T8_p, [P, NF, S], F8, "y1T8")
            qT = _T(qkT_p, [P, NF, SQ], BF16, "qT")
            kT = _T(qkT_p, [P, NF, S], BF16, "kT")

            # SP DMA queue order: x0-3, wq, xk4-7, wk, m, wv, wo, w1, w2.
            # Tiles 4-7 only feed LN1 -> fp8, so the host ships them bf16.
            xt_all = []
            for tt in range(NT):
                xt_all.append(xres[tt] if tt < NQ
                              else _T(xs_p, [P, D], BF16, "xs"))
            nc.sync.dma_start(out=xt_all[0][:, 0:SQ], in_=x3[0][:, 0:SQ])
            nc.sync.dma_start(out=xt_all[0][:, SQ:D], in_=x3[0][:, SQ:D])
            for tt in range(1, NQ):
                nc.sync.dma_start(out=xt_all[tt][:], in_=x3[tt])
            wq_t = [_T(wqk_p, [P, 2, D], F8, "wq") for _ in range(NJ)]
            for j in range(NJ):
                nc.sync.dma_start(out=wq_t[j][:], in_=t["wq"][j])
            for tt in range(NQ, NT):
                nc.sync.dma_start(out=xt_all[tt][:], in_=xk3[tt - NQ])
            wk_t = [_T(wqk_p, [P, 2, D], F8, "wk") for _ in range(NJ)]
            for j in range(NJ):
                nc.sync.dma_start(out=wk_t[j][:], in_=t["wk"][j])
            m2 = [_T(mk_p, [P, 2 * SQ], mybir.dt.uint8, "m2") for _ in range(NT // 2)]
            for kp in range(NT // 2):
                nc.sync.dma_start(out=m2[kp][:], in_=t["maskT2"][kp])
            wv_t = [_T(wv_p, [P, 2, D], F8, "wv") for _ in range(NJ)]
            for j in range(NJ):
                nc.sync.dma_start(out=wv_t[j][:], in_=t["wv"][j])

            def emit_ln1_stats(tt):
                return _ln_stats(nc, small_p, xt_all[tt][:], epst)

            def emit_ln1_apply(tt, mvrs, ps_tp):
                xt = xt_all[tt]
                mv, rs = mvrs
                yt = _T(y1_p, [P, D], BF16, "y1")
                nc.gpsimd.tensor_scalar(out=yt[:], in0=xt[:],
                                        scalar1=mv[:, 0:1], scalar2=rs[:],
                                        op0=ALU.subtract, op1=ALU.mult)
                for fc in range(NF):
                    pt = _T(ps_tp, [P, P], BF16, "pt")
                    nc.tensor.transpose(pt[:], yt[:, fc * P:(fc + 1) * P],
                                        ident[:])
                    dst = y1T8[:, fc, tt * P:(tt + 1) * P]
                    if fc < 6:
                        nc.scalar.copy(out=dst, in_=pt[:])
                    else:
                        nc.vector.tensor_copy(out=dst, in_=pt[:])

            from contextlib import ExitStack
            _p1 = ExitStack()
            ps_s2 = _p1.enter_context(tc.tile_pool(
                name=pfx + "ps_s2", bufs=2, space=bass.MemorySpace.PSUM))
            ps_qkv = _p1.enter_context(tc.tile_pool(
                name=pfx + "ps_qkv", bufs=1, space=bass.MemorySpace.PSUM))
            if True:
                def kT_chunk(fo, th):
                    tsl = slice(th * SQ, (th + 1) * SQ)
                    ps = _T(ps_qkv, [P, 2 * SQ], F32, "pqk")
                    half = ps[:, 0:SQ] if th == 0 else ps[:, SQ:2 * SQ]
                    for j in range(NJ):
                        nc.tensor.matmul(half,
                                         wk_t[j][:, :, fo * P:(fo + 1) * P],
                                         y1T8[:, 2 * j:2 * j + 2, tsl],
                                         start=(j == 0),
                                         stop=(not FL["bk"] and j == NJ - 1),
                                         perf_mode=DR)
                    if FL["bk"]:
                        nc.tensor.matmul(half,
                                         brow["bk"][:, fo * P:(fo + 1) * P],
                                         ones[:], start=False, stop=True)
                    nc.vector.tensor_copy(out=kT[:, fo, tsl], in_=half)

                def score2(h, kp):
                    """scores^T -> exp for one (head, kt-pair): E [128,1024]
                    covering key tiles 2kp, 2kp+1."""
                    fc, po = h // 2, (h % 2) * d
                    ps2 = _T(ps_s2, [P, 2 * SQ], F32, "ps2")
                    for sub in range(2):
                        kt = 2 * kp + sub
                        nc.tensor.matmul(ps2[:, sub * SQ:(sub + 1) * SQ],
                                         kT[po:po + d, fc, kt * P:(kt + 1) * P],
                                         qT[po:po + d, fc, :],
                                         start=True, stop=True)
                    e = _T(e_p, [P, 2 * SQ], BF16, "e")
                    nc.scalar.activation(out=e[:], in_=ps2[:], func=ACTF.Exp,
                                         scale=1.0 / (SCQ * SCK))
                    nc.gpsimd.tensor_mul(e[:], e[:], m2[kp][:])
                    return e

                V = [_T(V_p, [P, H, 2 * d], BF16, "V") for _ in range(NT)]

                def V_mm(kt):
                    # V token-major, heads interleaved d v-cols then d 16-cols
                    nc.gpsimd.memset(V[kt][:, :, d:], SCV)
                    ps = _T(ps_s2, [P, 2 * SQ], F32, "ps2")
                    for fh in range(2):
                        half = ps[:, fh * SQ:(fh + 1) * SQ]
                        for j in range(NJ):
                            nc.tensor.matmul(half,
                                             y1T8[:, 2 * j:2 * j + 2,
                                                  kt * P:(kt + 1) * P],
                                             wv_t[j][:, :, fh * SQ:(fh + 1) * SQ],
                                             start=(j == 0),
                                             stop=(not FL["bv"] and j == NJ - 1),
                                             perf_mode=DR)
                        if FL["bv"]:
                            nc.tensor.matmul(half, ones[:, 0:P],
                                             brow["bv"][:, fh * SQ:(fh + 1) * SQ],
                                             start=False, stop=True)
                    nc.vector.tensor_copy(
                        out=V[kt][:, :, 0:d],
                        in_=ps[:].rearrange("p (a b) -> p a b", b=d))

                e01 = {0: [], 1: []}
                with tc.tile_pool(name=pfx + "ps_tp", bufs=2,
                                  space=bass.MemorySpace.PSUM) as ps_tp:
                    for tt in range(NQ):
                        emit_ln1_apply(tt, emit_ln1_stats(tt), ps_tp)

                    # q^T: fp8 DoubleRow, fo pairs share a PSUM buf (own 512 q)
                    for fe in range(0, NF, 2):
                        ps = _T(ps_qkv, [P, 2 * SQ], F32, "pqk")
                        for sub in range(2):
                            fo = fe + sub
                            half = ps[:, sub * SQ:(sub + 1) * SQ]
                            for j in range(NJ):
                                nc.tensor.matmul(half,
                                                 wq_t[j][:, :, fo * P:(fo + 1) * P],
                                                 y1T8[:, 2 * j:2 * j + 2, 0:SQ],
                                                 start=(j == 0),
                                                 stop=(not FL["bq"] and j == NJ - 1),
                                                 perf_mode=DR)
                            if FL["bq"]:
                                nc.tensor.matmul(half,
                                                 brow["bq"][:, fo * P:(fo + 1) * P],
                                                 ones[:], start=False, stop=True)
                        nc.vector.tensor_copy(
                            out=qT[:, fe:fe + 2, :],
                            in_=ps[:].rearrange("p (a b) -> p a b", b=SQ))

                    kT_chunk(0, 0)
                    kT_chunk(1, 0)
                    # stats (and their ACT sqrts) for tiles 4-7 queue before
                    # the first exp so the Sqrt/Exp tables load exactly once
                    mvrs = {tt: emit_ln1_stats(tt) for tt in range(NQ, NT)}
                    # pair-0 scores for the first two kt-pairs start while the
                    # second half of the sequence is still loading/normalizing
                    for kp in (0, 1):
                        for h in (0, 1):
                            e01[h].append(score2(h, kp))
                    emit_ln1_apply(NQ, mvrs[NQ], ps_tp)
                    emit_ln1_apply(NQ + 1, mvrs[NQ + 1], ps_tp)
                    V_mm(0)
                    emit_ln1_apply(NQ + 2, mvrs[NQ + 2], ps_tp)
                    V_mm(1)
                    emit_ln1_apply(NQ + 3, mvrs[NQ + 3], ps_tp)
                    V_mm(2)
                    V_mm(3)

                kT_chunk(0, 1)
                kT_chunk(1, 1)
                for kt in range(4, NT):
                    V_mm(kt)
                for kp in (2, 3):
                    for h in (0, 1):
                        e01[h].append(score2(h, kp))
                kT_chunk(2, 0)
                kT_chunk(2, 1)

                with tc.tile_pool(name=pfx + "ps_ctx", bufs=2,
                                  space=bass.MemorySpace.PSUM) as ps_ctx:
                    def ctx_mm(pcs, h, kp, e):
                        for sub in range(2):
                            kt = 2 * kp + sub
                            nc.tensor.matmul(pcs[:], V[kt][:, h, :],
                                             e[:, sub * SQ:(sub + 1) * SQ],
                                             start=(kt == 0), stop=(kt == NT - 1),
                                             skip_group_check=True)

                    def ctx_norm(h, pcs):
                        fco, po = h // 2, (h % 2) * d
                        rb = _T(small_p, [d, SQ], F32, "rb")
                        nc.vector.reciprocal(rb[:], pcs[d:2 * d, :])
                        nc.vector.tensor_mul(ctxT8[po:po + 64, fco, :],
                                             pcs[0:d, :], rb[:])

                    pcs0 = {h: _T(ps_ctx, [P, SQ], F32, "pc") for h in (0, 1)}
                    for kp in range(NT // 2):
                        for h in (0, 1):
                            ctx_mm(pcs0[h], h, kp, e01[h][kp])
                    for h in (0, 1):
                        ctx_norm(h, pcs0[h])
                    del e01, pcs0

                    LAG = 2
                    for fc in range(1, NF):
                        pair = (2 * fc, 2 * fc + 1)
                        es = {h: [] for h in pair}
                        pcs = {h: _T(ps_ctx, [P, SQ], F32, "pc") for h in pair}
                        for kp in range(NT // 2):
                            for h in pair:
                                es[h].append(score2(h, kp))
                            if kp == 0 and fc + 2 < NF:
                                kT_chunk(fc + 2, 0)
                                kT_chunk(fc + 2, 1)
                            if kp >= LAG:
                                for h in pair:
                                    ctx_mm(pcs[h], h, kp - LAG, es[h][kp - LAG])
                        for kp in range(NT // 2 - LAG, NT // 2):
                            for h in pair:
                                ctx_mm(pcs[h], h, kp, es[h][kp])
                        for h in pair:
                            ctx_norm(h, pcs[h])

            _p1.close()

        # ================= phase 2: Wo, LN2, FFN =================
        with (
            tc.tile_pool(name=pfx + "wo", bufs=2 * NJ) as wo_p,
            tc.tile_pool(name=pfx + "x2", bufs=NQ) as x2_p,
            tc.tile_pool(name=pfx + "y2", bufs=2) as y2_p,
            tc.tile_pool(name=pfx + "y2T", bufs=1) as y2T_p,
            tc.tile_pool(name=pfx + "x2t", bufs=2) as x2t_p,
        ):
            wo_t = [_T(wo_p, [P, 2, SQ], F8, "woW") for _ in range(2 * NJ)]
            for i in range(2 * NJ):
                nc.sync.dma_start(out=wo_t[i][:], in_=t["wo"][i])
            w1a_t = [_T(w1a_p, [P, 2, F], F8, "w1a") for _ in range(NA1 // 2)]
            for j in range(NA1 // 2):
                nc.sync.dma_start(out=w1a_t[j][:], in_=t["w1a"][j])
            w1_t = [_T(w1_p, [P, (NF - NA1) * P], BF16, "w1s")
                    for _ in range(NF1)]
            for f1 in range(NF1):
                nc.sync.dma_start(out=w1_t[f1][:], in_=t["w1b"][f1])
            w2a_t = [_T(wh_p, [P, 2, SQ], F8, "w2a")
                     for _ in range(2 * (NA2 // 2))]
            for i in range(2 * (NA2 // 2)):
                nc.sync.dma_start(out=w2a_t[i][:], in_=t["w2a"][i])
            w2_t = [_T(wh_p, [P, SQ], BF16, "wh")
                    for _ in range(2 * (NF1 - NA2))]
            for i in range(2 * (NF1 - NA2)):
                nc.sync.dma_start(out=w2_t[i][:], in_=t["w2b"][i])

            x2 = [_T(x2_p, [P, D], F32, "x2") for _ in range(NQ)]
            y2T8 = _T(y2T_p, [P, NA1, SQ], F8, "y2T8")
            y2T = _T(y2T_p, [P, NF - NA1, SQ], BF16, "y2T")

            with (
                tc.tile_pool(name=pfx + "ps_wo", bufs=2,
                             space=bass.MemorySpace.PSUM) as ps_wo,
                tc.tile_pool(name=pfx + "ps_tp2", bufs=2,
                             space=bass.MemorySpace.PSUM) as ps_tp2,
            ):
                def emit_wo(qt, ps):
                    for dh in range(2):
                        half = ps[:, dh * SQ:(dh + 1) * SQ]
                        for j in range(NJ):
                            nc.tensor.matmul(
                                half,
                                ctxT8[:, 2 * j:2 * j + 2, qt * P:(qt + 1) * P],
                                wo_t[dh * NJ + j][:],
                                start=(j == 0), stop=(j == NJ - 1),
                                perf_mode=DR)

                def emit_ln2(qt, ps):
                    xt2 = _T(x2t_p, [P, D], F32, "x2t")
                    nc.scalar.activation(out=xt2[:], in_=ps[:], func=ACTF.Copy,
                                         scale=1.0 / SCO)
                    nc.vector.tensor_add(x2[qt][:], xt2[:], xres[qt][:])
                    if FL["bo"]:
                        nc.vector.tensor_add(x2[qt][:], x2[qt][:], bo_b[:])
                    yt = _T(y2_p, [P, D], BF16, "y2")
                    mv, rs = _ln_stats(nc, small_p, x2[qt][:], epst)
                    nc.vector.tensor_scalar(out=yt[:], in0=x2[qt][:],
                                            scalar1=mv[:, 0:1], scalar2=rs[:],
                                            op0=ALU.subtract, op1=ALU.mult)
                    return yt

                def emit_y2T(qt, yt):
                    for fc in range(NF):
                        pt = _T(ps_tp2, [P, P], BF16, "pt2")
                        nc.tensor.transpose(pt[:], yt[:, fc * P:(fc + 1) * P],
                                            ident[:])
                        dst = (y2T8[:, fc, qt * P:(qt + 1) * P] if fc < NA1
                               else y2T[:, fc - NA1, qt * P:(qt + 1) * P])
                        nc.scalar.copy(out=dst, in_=pt[:])

                wops = [_T(ps_wo, [P, D], F32, "pwo") for _ in range(2)]
                emit_wo(0, wops[0])
                emit_wo(1, wops[1])
                y0 = emit_ln2(0, wops[0])
                y1_ = emit_ln2(1, wops[1])
                wops2 = [_T(ps_wo, [P, D], F32, "pwo") for _ in range(2)]
                emit_wo(2, wops2[0])
                emit_y2T(0, y0)
                emit_wo(3, wops2[1])
                emit_y2T(1, y1_)
                y2_ = emit_ln2(2, wops2[0])
                y3 = emit_ln2(3, wops2[1])
                emit_y2T(2, y2_)
                emit_y2T(3, y3)

            # FFN1 (bf16): h^T = gelu(W1' y2 + b1'), f1 pairs share a PSUM buf
            with (
                tc.tile_pool(name=pfx + "hT", bufs=1) as hT_p,
                tc.tile_pool(name=pfx + "xo", bufs=NQ) as xo_p,
                tc.tile_pool(name=pfx + "ps_f1", bufs=2,
                             space=bass.MemorySpace.PSUM) as ps_f1,
                tc.tile_pool(name=pfx + "ps_4", bufs=4,
                             space=bass.MemorySpace.PSUM) as ps_4,
            ):
                hT8 = _T(hT_p, [P, NA2, SQ], F8, "hT8")
                hT = _T(hT_p, [P, NF1 - NA2, SQ], BF16, "hT")
                for fe in range(0, NF1, 2):
                    ps = _T(ps_f1, [P, 2 * SQ], F32, "pf1")
                    for sub in range(2):
                        f1 = fe + sub
                        half = ps[:, sub * SQ:(sub + 1) * SQ]
                        for j in range(NA1 // 2):
                            nc.tensor.matmul(half,
                                             w1a_t[j][:, :, f1 * P:(f1 + 1) * P],
                                             y2T8[:, 2 * j:2 * j + 2, :],
                                             start=(j == 0), stop=False,
                                             perf_mode=DR)
                        nb = NF - NA1
                        for fi in range(nb):
                            nc.tensor.matmul(half,
                                             w1_t[f1][:, fi * P:(fi + 1) * P],
                                             y2T[:, fi, :],
                                             start=False,
                                             stop=(not FL["b1"] and fi == nb - 1))
                        if FL["b1"]:
                            nc.tensor.matmul(half,
                                             brow["b1"][:, f1 * P:(f1 + 1) * P],
                                             ones[:], start=False, stop=True)
                    out_sl = (hT8[:, fe:fe + 2, :] if fe < NA2
                              else hT[:, fe - NA2:fe - NA2 + 2, :])
                    nc.scalar.activation(out=out_sl.rearrange(
                        "p a b -> p (a b)"), in_=ps[:], func=ACTF.Gelu,
                        scale=1.0 / SC1)

                # FFN2 + residual: out = x2 + h @ W2 + b2
                xout = [_T(xo_p, [P, D], F32, "xo") for _ in range(NQ)]
                nb2 = NF1 - NA2
                for dh in range(2):
                    dsl = slice(dh * SQ, (dh + 1) * SQ)
                    ps4 = [_T(ps_4, [P, SQ], F32, "p4") for _ in range(NQ)]
                    for j in range(NA2 // 2):
                        for qt in range(NQ):
                            nc.tensor.matmul(ps4[qt][:],
                                             hT8[:, 2 * j:2 * j + 2,
                                                 qt * P:(qt + 1) * P],
                                             w2a_t[dh * (NA2 // 2) + j][:],
                                             start=(j == 0), stop=False,
                                             perf_mode=DR)
                    for f1 in range(nb2):
                        for qt in range(NQ):
                            nc.tensor.matmul(ps4[qt][:],
                                             hT[:, f1, qt * P:(qt + 1) * P],
                                             w2_t[dh * nb2 + f1][:],
                                             start=False,
                                             stop=(not FL["b2"] and f1 == nb2 - 1))
                    for qt in range(NQ):
                        if FL["b2"]:
                            nc.tensor.matmul(ps4[qt][:], ones[:, 0:P],
                                             brow["b2"][:, dsl],
                                             start=False, stop=True)
                        xsc = _T(x2t_p, [P, SQ], F32, "xsc")
                        if qt % 2 == 0:
                            nc.vector.tensor_scalar(out=xsc[:], in0=ps4[qt][:],
                                                    scalar1=1.0 / SC2,
                                                    scalar2=None, op0=ALU.mult)
                        else:
                            nc.scalar.activation(out=xsc[:], in_=ps4[qt][:],
                                                 func=ACTF.Copy, scale=1.0 / SC2)
                        nc.vector.tensor_add(xout[qt][:, dsl], xsc[:],
                                             x2[qt][:, dsl])
                        nc.sync.dma_start(out=out3[qt][:, dsl],
                                          in_=xout[qt][:, dsl])


_NC = {}
_ALL_FLAGS = ("bq", "bk", "bv", "bo", "b1", "b2")


def _get_nc(flags=None, reps=1):
    if flags is None:
        flags = {k: True for k in _ALL_FLAGS}
    key = (tuple(sorted(flags.items())), reps)
    if key not in _NC:
        _NC[key] = _build_program(dict(flags), reps=reps)
    return _NC[key]


def _q8(w, scale):
    return np.clip(w * scale, -224.0, 224.0).astype(E4)


def _prep_inputs(inputs):
    """Host-side folding + per-core shard maps."""
    x = np.asarray(inputs["x"], np.float32)
    mask = np.asarray(inputs["mask"], np.float32)
    g1 = np.asarray(inputs["ln1_g"], np.float32)
    b1n = np.asarray(inputs["ln1_b"], np.float32)
    g2 = np.asarray(inputs["ln2_g"], np.float32)
    b2n = np.asarray(inputs["ln2_b"], np.float32)
    Wq = np.asarray(inputs["Wq"], np.float32); bq = np.asarray(inputs["bq"], np.float32)
    Wk = np.asarray(inputs["Wk"], np.float32); bk = np.asarray(inputs["bk"], np.float32)
    Wv = np.asarray(inputs["Wv"], np.float32); bv = np.asarray(inputs["bv"], np.float32)
    Wo = np.asarray(inputs["Wo"], np.float32); bo = np.asarray(inputs["bo"], np.float32)
    W1 = np.asarray(inputs["W1"], np.float32); b1 = np.asarray(inputs["b1"], np.float32)
    W2 = np.asarray(inputs["W2"], np.float32); b2 = np.asarray(inputs["b2"], np.float32)

    scale = d ** -0.5
    # fold LN gains/biases (and q scale) into the projection weights
    Wq_e = (g1[:, None] * Wq) * scale
    bq_e = (b1n @ Wq + bq) * scale
    Wk_e = g1[:, None] * Wk
    bk_e = b1n @ Wk + bk
    Wv_e = g1[:, None] * Wv
    bv_e = b1n @ Wv + bv
    W1_e = g2[:, None] * W1
    b1_e = b2n @ W1 + b1

    def tile_dr(W, sc):
        # [Din, Dout] -> [NJ(j), P, 2(i), Dout] fp8, f_in = (2j+i)*128 + p
        return np.ascontiguousarray(
            _q8(W, sc).reshape(NJ, 2, P, D).transpose(0, 2, 1, 3))

    wq_h = tile_dr(Wq_e, SCQ)
    wk_h = tile_dr(Wk_e, SCK)
    wv_h = tile_dr(Wv_e, SCV)
    # Wo -> [2(dh)*NJ(j), P, 2(i), SQ] fp8
    wo_h = np.ascontiguousarray(
        _q8(Wo, SCO).reshape(NJ, 2, P, 2, SQ).transpose(3, 0, 2, 1, 4)
        .reshape(2 * NJ, P, 2, SQ))
    # FFN1: f_in chunks [0, NA1) in fp8 DoubleRow pairs, rest bf16 (both *SC1)
    ka = NA1 * P
    w1a_h = np.ascontiguousarray(
        _q8(W1_e[:ka], SC1).reshape(NA1 // 2, 2, P, F).transpose(0, 2, 1, 3))
    w1b_h = np.ascontiguousarray(
        (W1_e[ka:] * SC1).reshape(NF - NA1, P, NF1, P)
        .transpose(2, 1, 0, 3).reshape(NF1, P, (NF - NA1) * P)).astype(BF)
    # FFN2: f1 chunks [0, NA2) in fp8 DoubleRow pairs, rest bf16 (both *SC2)
    kb = NA2 * P
    w2a_h = np.ascontiguousarray(
        _q8(W2[:kb], SC2).reshape(NA2 // 2, 2, P, 2, SQ)
        .transpose(3, 0, 2, 1, 4).reshape(2 * (NA2 // 2), P, 2, SQ))
    w2b_h = np.ascontiguousarray(
        (W2[kb:] * SC2).reshape(NF1 - NA2, P, 2, SQ)
        .transpose(2, 0, 1, 3).reshape(2 * (NF1 - NA2), P, SQ)).astype(BF)

    flags = {
        "bq": bool(np.any(bq_e)), "bk": bool(np.any(bk_e)),
        "bv": bool(np.any(bv_e)), "bo": bool(np.any(bo)),
        "b1": bool(np.any(b1_e)), "b2": bool(np.any(b2)),
    }
    shared = {
        "ident": np.eye(P, dtype=BF),
        "wq": wq_h, "wk": wk_h, "wv": wv_h, "wo": wo_h,
        "w1a": w1a_h, "w1b": w1b_h, "w2a": w2a_h, "w2b": w2b_h,
        "bq": (bq_e * SCQ).reshape(1, D).astype(BF),
        "bk": (bk_e * SCK).reshape(1, D).astype(BF),
        "bv": (bv_e * SCV).reshape(1, D).astype(BF),
        "bo": bo.reshape(1, D).astype(np.float32),
        "b1": (b1_e * SC1).reshape(1, F).astype(BF),
        "b2": (b2 * SC2).reshape(1, D).astype(BF),
    }

    in_maps = []
    for c in range(8):
        b, hf = c // 2, c % 2
        x_rot = np.ascontiguousarray(np.roll(x[b], -SQ * hf, axis=0))
        mq = mask[b, 0, SQ * hf:SQ * (hf + 1), :]          # [512 q, 1024 k]
        mT = np.roll(mq.T, -SQ * hf, axis=0)               # [1024 k, 512 q]
        # kt-pair layout: [4, 128, 1024] with pair tiles side by side
        mT2 = np.ascontiguousarray(
            mT.reshape(NT // 2, 2, P, SQ).transpose(0, 2, 1, 3)
            .reshape(NT // 2, P, 2 * SQ)).astype(np.uint8)
        m = dict(shared)
        m["x"] = np.ascontiguousarray(x_rot[:SQ])
        m["xk"] = np.ascontiguousarray(x_rot[SQ:]).astype(BF)
        m["maskT2"] = mT2
        in_maps.append(m)
    return in_maps, flags


def run(inputs, trace=False, **kw):
    in_maps, flags = _prep_inputs(inputs)
    nc = _get_nc(flags)
    res = run_bass_kernel_spmd(nc, in_maps, core_ids=list(range(8)),
                               trace=trace, **kw)
    out = np.empty((B, S, D), np.float32)
    for c in range(8):
        b, hf = c // 2, c % 2
        out[b, SQ * hf:SQ * (hf + 1), :] = res.results[c]["out"]
    return out, res


def kernel(**inputs) -> np.ndarray:
    out, _ = run(inputs, trace=False)
    return out


# revision 26
# speedup vs baseline: 1.7576x; 1.7576x over previous
"""Trainium2 Bass kernel for a dense transformer block (B=4,S=1024,D=1024,F=4096,H=16).

Sharding: 8 cores = (batch b in 0..3) x (seq half). Pure SPMD, no collectives:
the host rotates each core's tokens so its 512 query rows are always rows
0..511 of the rotated sequence; K/V cover the full (rotated) sequence.

v2: fp8e4m3 DoubleRow matmuls (0.5 cyc/row) for the QKV projections and Wo.
  - y1 = LN1(x) transposed to a single interleaved fp8 tile y1T8 [128, 8, 1024]
    so each DoubleRow matmul contracts 256 features (2 chunks of 128).
  - weights are pre-scaled on host (Wq*128, Wk*16, Wv*16, Wo*16) to lift the
    0.02-scale entries out of fp8 subnormal range; the score exp absorbs
    1/(128*16) via its activation scale, the softmax ones-column is set to 16
    to absorb V's scale, and an ACT copy with scale=1/16 rescales Wo's output.
  - scores/ctx and the FFN stay bf16 (fp8 there costs too much accuracy).
Scheduling: x DMAs lead the SP queue (weights follow, FFN weights prefetched
through outer-scope stream pools), LN runs split 0-3 / 4-7 so qT and the kT
half-chunks start as soon as their token tiles are normalized, exps are
batched over kt-pairs ([128,1024] PSUM reads), mask-multiply + LN1 apply +
V-ones memsets + part of the copies/adds run on GpSimd(Pool).
The softmax row-sum rides the ctx matmul as 16.0-columns of V_aug; the
per-(b,h) additive attn_bias is constant over q and k so softmax cancels it.
"""

import numpy as np
import ml_dtypes

import concourse.bass as bass
import concourse.mybir as mybir
import concourse.tile as tile
from concourse import bacc
from concourse.bass_utils import run_bass_kernel_spmd

F32 = mybir.dt.float32
BF16 = mybir.dt.bfloat16
F8 = mybir.dt.float8e4
BF = ml_dtypes.bfloat16
E4 = ml_dtypes.float8_e4m3

B, S, D, F, H = 4, 1024, 1024, 4096, 16
d = D // H          # 64
P = 128             # partitions
SQ = 512            # queries per core
EPS = 1e-5
NT = S // P         # 8 token tiles (full seq)
NQ = SQ // P        # 4 query tiles
NF = D // P         # 8 feature chunks
NJ = NF // 2        # 4 fp8 contraction pairs
NF1 = F // P        # 32 ffn chunks

SCQ = 128.0         # host pre-scale on Wq (includes d^-0.5 fold)
SCK = 16.0          # host pre-scale on Wk
SCV = 16.0          # host pre-scale on Wv (absorbed by 16.0 ones-column)
SCO = 16.0          # host pre-scale on Wo (rescaled by ACT copy 1/16)
SC1 = 16.0          # pre-scale on W1 (both halves; gelu scale absorbs 1/16)
SC2 = 16.0          # pre-scale on W2 (both halves; ACT copy absorbs 1/16)
NA1 = 4             # f_in chunks of FFN1 run in fp8 (alpha = NA1/NF)
NA2 = 2             # f1 chunks of FFN2 run in fp8 (alpha = NA2/NF1)

AX = mybir.AxisListType
ALU = mybir.AluOpType
ACTF = mybir.ActivationFunctionType
DR = mybir.MatmulPerfMode.DoubleRow


def _T(pool, shape, dtype, tag):
    return pool.tile(shape, dtype, name=tag, tag=tag)


def _pbcast(ap, p):
    """Partition-broadcast a [1, N] DRAM AP to [p, N]."""
    return bass.AP(tensor=ap.tensor, offset=ap.offset, ap=[[0, p]] + list(ap.ap[1:]))


def _build_program(FL, reps=1):
    nc = bacc.Bacc("TRN2", target_bir_lowering=False, debug=False)

    t = {}
    t["x"] = nc.dram_tensor("x", [SQ, D], F32, kind="ExternalInput").ap()
    t["xk"] = nc.dram_tensor("xk", [SQ, D], BF16, kind="ExternalInput").ap()
    t["maskT2"] = nc.dram_tensor("maskT2", [NT // 2, P, 2 * SQ], mybir.dt.uint8,
                                 kind="ExternalInput").ap()
    t["wq"] = nc.dram_tensor("wq", [NJ, P, 2, D], F8, kind="ExternalInput").ap()
    t["wk"] = nc.dram_tensor("wk", [NJ, P, 2, D], F8, kind="ExternalInput").ap()
    t["wv"] = nc.dram_tensor("wv", [NJ, P, 2, D], F8, kind="ExternalInput").ap()
    t["wo"] = nc.dram_tensor("wo", [2 * NJ, P, 2, SQ], F8, kind="ExternalInput").ap()
    t["w1a"] = nc.dram_tensor("w1a", [NA1 // 2, P, 2, F], F8,
                              kind="ExternalInput").ap()
    t["w1b"] = nc.dram_tensor("w1b", [NF1, P, (NF - NA1) * P], BF16,
                              kind="ExternalInput").ap()
    t["w2a"] = nc.dram_tensor("w2a", [2 * (NA2 // 2), P, 2, SQ], F8,
                              kind="ExternalInput").ap()
    t["w2b"] = nc.dram_tensor("w2b", [2 * (NF1 - NA2), P, SQ], BF16,
                              kind="ExternalInput").ap()
    t["bq"] = nc.dram_tensor("bq", [1, D], BF16, kind="ExternalInput").ap()
    t["bk"] = nc.dram_tensor("bk", [1, D], BF16, kind="ExternalInput").ap()
    t["bv"] = nc.dram_tensor("bv", [1, D], BF16, kind="ExternalInput").ap()
    t["bo"] = nc.dram_tensor("bo", [1, D], F32, kind="ExternalInput").ap()
    t["b1"] = nc.dram_tensor("b1", [1, F], BF16, kind="ExternalInput").ap()
    t["b2"] = nc.dram_tensor("b2", [1, D], BF16, kind="ExternalInput").ap()
    t["ident"] = nc.dram_tensor("ident", [P, P], BF16, kind="ExternalInput").ap()
    t["out"] = nc.dram_tensor("out", [SQ, D], F32, kind="ExternalOutput").ap()

    with tile.TileContext(nc) as tc:
        for rep in range(reps):
            _trace(nc, tc, t, FL, pfx=f"r{rep}_" if reps > 1 else "")
    nc.compile()
    return nc


def _ln_stats(nc, pool, x_ap, epst):
    """Return [P,1] mean and rstd tiles for a [P, 1024] input."""
    st = _T(pool, [P, 2, 6], F32, "st")
    xr = x_ap.rearrange("p (a b) -> p a b", b=512)
    for sg in range(2):
        nc.vector.bn_stats(out=st[:, sg, :], in_=xr[:, sg, :])
    mv = _T(pool, [P, 2], F32, "mv")
    nc.vector.bn_aggr(out=mv[:], in_=st[:])
    rs = _T(pool, [P, 1], F32, "rs")
    nc.scalar.activation(out=rs[:], in_=mv[:, 1:2], func=ACTF.Sqrt,
                         bias=epst[:], scale=1.0)
    nc.vector.reciprocal(out=rs[:], in_=rs[:])
    return mv, rs


def _trace(nc, tc, t, FL, pfx=""):
    x3 = t["x"].rearrange("(n p) c -> n p c", p=P)
    xk3 = t["xk"].rearrange("(n p) c -> n p c", p=P)
    out3 = t["out"].rearrange("(n p) c -> n p c", p=P)

    with (
        tc.tile_pool(name=pfx + "const", bufs=1) as const_p,
        tc.tile_pool(name=pfx + "small", bufs=4) as small_p,
        tc.tile_pool(name=pfx + "xres", bufs=NQ) as xres_p,
        tc.tile_pool(name=pfx + "ctxT", bufs=1) as ctxT_p,
        tc.tile_pool(name=pfx + "w1a", bufs=NA1 // 2) as w1a_p,
        tc.tile_pool(name=pfx + "w1s", bufs=6) as w1_p,
        tc.tile_pool(name=pfx + "w2s", bufs=10) as wh_p,
    ):
        # ---- constants ----
        ident = _T(const_p, [P, P], BF16, "ident")
        nc.gpsimd.dma_start(out=ident[:], in_=t["ident"])
        if any(FL[k] for k in ("bq", "bk", "bv", "b1", "b2")):
            ones = _T(const_p, [1, SQ], BF16, "ones")
            nc.vector.memset(ones[:], 1.0)
        epst = _T(const_p, [P, 1], F32, "epst")
        nc.vector.memset(epst[:], EPS)
        actws = _T(const_p, [P, 1], F32, "actws")

        def act_warm(func):
            # dependency-free dummy that hides the 1.3us ACT table load
            nc.scalar.activation(out=actws[:], in_=epst[:], func=func)
        if FL["bo"]:
            bo_b = _T(const_p, [P, D], F32, "bo_b")
            nc.sync.dma_start(out=bo_b[:], in_=_pbcast(t["bo"], P))
        brow = {}
        for nm in ("bq", "bk", "bv", "b1", "b2"):
            if not FL[nm]:
                continue
            sz = F if nm == "b1" else D
            brow[nm] = _T(const_p, [1, sz], BF16, f"brow_{nm}")
            nc.sync.dma_start(out=brow[nm][:], in_=t[nm])

        xres = [_T(xres_p, [P, D], F32, "xres") for _ in range(NQ)]
        ctxT8 = _T(ctxT_p, [P, NF, SQ], F8, "ctxT8")

        # ================= phase 1: LN1, y1T8, QKV, attention =================
        with (
            tc.tile_pool(name=pfx + "y1T8", bufs=1) as y1T8_p,
            tc.tile_pool(name=pfx + "qkT", bufs=1) as qkT_p,
            tc.tile_pool(name=pfx + "wqk", bufs=NJ) as wqk_p,
            tc.tile_pool(name=pfx + "wv", bufs=NJ) as wv_p,
            tc.tile_pool(name=pfx + "Vp", bufs=NT) as V_p,
            tc.tile_pool(name=pfx + "mk", bufs=NT // 2) as mk_p,
            tc.tile_pool(name=pfx + "es", bufs=10) as e_p,
            tc.tile_pool(name=pfx + "xs", bufs=4) as xs_p,
            tc.tile_pool(name=pfx + "y1", bufs=3) as y1_p,
        ):
            y1T8 = _T(y1T8_p, [P, NF, S], F8, "y1T8")
            qT = _T(qkT_p, [P, NF, SQ], BF16, "qT")
            kT = _T(qkT_p, [P, NF, S], BF16, "kT")

            # SP DMA queue order: x0-3, wq, xk4-7, wk, m, wv, wo, w1, w2.
            # Tiles 4-7 only feed LN1 -> fp8, so the host ships them bf16.
            xt_all = []
            for tt in range(NT):
                xt_all.append(xres[tt] if tt < NQ
                              else _T(xs_p, [P, D], BF16, "xs"))
            act_warm(ACTF.Sqrt)
            nc.sync.dma_start(out=xt_all[0][:, 0:SQ], in_=x3[0][:, 0:SQ])
            nc.sync.dma_start(out=xt_all[0][:, SQ:D], in_=x3[0][:, SQ:D])
            for tt in range(1, NQ):
                nc.sync.dma_start(out=xt_all[tt][:], in_=x3[tt])
            wq_t = [_T(wqk_p, [P, 2, D], F8, "wq") for _ in range(NJ)]
            for j in range(NJ):
                nc.sync.dma_start(out=wq_t[j][:], in_=t["wq"][j])
            for tt in range(NQ, NT):
                nc.sync.dma_start(out=xt_all[tt][:], in_=xk3[tt - NQ])
            wk_t = [_T(wqk_p, [P, 2, D], F8, "wk") for _ in range(NJ)]
            for j in range(NJ):
                nc.sync.dma_start(out=wk_t[j][:], in_=t["wk"][j])
            m2 = [_T(mk_p, [P, 2 * SQ], mybir.dt.uint8, "m2") for _ in range(NT // 2)]
            for kp in range(NT // 2):
                nc.sync.dma_start(out=m2[kp][:], in_=t["maskT2"][kp])
            wv_t = [_T(wv_p, [P, 2, D], F8, "wv") for _ in range(NJ)]
            for j in range(NJ):
                nc.sync.dma_start(out=wv_t[j][:], in_=t["wv"][j])

            def emit_ln1_stats(tt):
                return _ln_stats(nc, small_p, xt_all[tt][:], epst)

            def emit_ln1_apply(tt, mvrs, ps_tp):
                xt = xt_all[tt]
                mv, rs = mvrs
                yt = _T(y1_p, [P, D], BF16, "y1")
                nc.gpsimd.tensor_scalar(out=yt[:], in0=xt[:],
                                        scalar1=mv[:, 0:1], scalar2=rs[:],
                                        op0=ALU.subtract, op1=ALU.mult)
                for fc in range(NF):
                    pt = _T(ps_tp, [P, P], BF16, "pt")
                    nc.tensor.transpose(pt[:], yt[:, fc * P:(fc + 1) * P],
                                        ident[:])
                    dst = y1T8[:, fc, tt * P:(tt + 1) * P]
                    if fc < 6:
                        nc.scalar.copy(out=dst, in_=pt[:])
                    else:
                        nc.vector.tensor_copy(out=dst, in_=pt[:])

            from contextlib import ExitStack
            _p1 = ExitStack()
            ps_s2 = _p1.enter_context(tc.tile_pool(
                name=pfx + "ps_s2", bufs=2, space=bass.MemorySpace.PSUM))
            ps_qkv = _p1.enter_context(tc.tile_pool(
                name=pfx + "ps_qkv", bufs=1, space=bass.MemorySpace.PSUM))
            if True:
                def kT_chunk(fo, th):
                    tsl = slice(th * SQ, (th + 1) * SQ)
                    ps = _T(ps_qkv, [P, 2 * SQ], F32, "pqk")
                    half = ps[:, 0:SQ] if th == 0 else ps[:, SQ:2 * SQ]
                    for j in range(NJ):
                        nc.tensor.matmul(half,
                                         wk_t[j][:, :, fo * P:(fo + 1) * P],
                                         y1T8[:, 2 * j:2 * j + 2, tsl],
                                         start=(j == 0),
                                         stop=(not FL["bk"] and j == NJ - 1),
                                         perf_mode=DR)
                    if FL["bk"]:
                        nc.tensor.matmul(half,
                                         brow["bk"][:, fo * P:(fo + 1) * P],
                                         ones[:], start=False, stop=True)
                    nc.vector.tensor_copy(out=kT[:, fo, tsl], in_=half)

                def score2(h, kp):
                    """scores^T -> exp for one (head, kt-pair): E [128,1024]
                    covering key tiles 2kp, 2kp+1."""
                    fc, po = h // 2, (h % 2) * d
                    ps2 = _T(ps_s2, [P, 2 * SQ], F32, "ps2")
                    for sub in range(2):
                        kt = 2 * kp + sub
                        nc.tensor.matmul(ps2[:, sub * SQ:(sub + 1) * SQ],
                                         kT[po:po + d, fc, kt * P:(kt + 1) * P],
                                         qT[po:po + d, fc, :],
                                         start=True, stop=True)
                    e = _T(e_p, [P, 2 * SQ], BF16, "e")
                    nc.scalar.activation(out=e[:], in_=ps2[:], func=ACTF.Exp,
                                         scale=1.0 / (SCQ * SCK))
                    nc.gpsimd.tensor_mul(e[:], e[:], m2[kp][:])
                    return e

                V = [_T(V_p, [P, H, 2 * d], BF16, "V") for _ in range(NT)]

                def V_mm(kt):
                    # V token-major, heads interleaved d v-cols then d 16-cols
                    nc.gpsimd.memset(V[kt][:, :, d:], SCV)
                    ps = _T(ps_s2, [P, 2 * SQ], F32, "ps2")
                    for fh in range(2):
                        half = ps[:, fh * SQ:(fh + 1) * SQ]
                        for j in range(NJ):
                            nc.tensor.matmul(half,
                                             y1T8[:, 2 * j:2 * j + 2,
                                                  kt * P:(kt + 1) * P],
                                             wv_t[j][:, :, fh * SQ:(fh + 1) * SQ],
                                             start=(j == 0),
                                             stop=(not FL["bv"] and j == NJ - 1),
                                             perf_mode=DR)
                        if FL["bv"]:
                            nc.tensor.matmul(half, ones[:, 0:P],
                                             brow["bv"][:, fh * SQ:(fh + 1) * SQ],
                                             start=False, stop=True)
                    nc.vector.tensor_copy(
                        out=V[kt][:, :, 0:d],
                        in_=ps[:].rearrange("p (a b) -> p a b", b=d))

                e01 = {0: [], 1: []}
                with tc.tile_pool(name=pfx + "ps_tp", bufs=2,
                                  space=bass.MemorySpace.PSUM) as ps_tp:
                    for tt in range(NQ):
                        emit_ln1_apply(tt, emit_ln1_stats(tt), ps_tp)

                    # q^T: fp8 DoubleRow, fo pairs share a PSUM buf (own 512 q)
                    for fe in range(0, NF, 2):
                        ps = _T(ps_qkv, [P, 2 * SQ], F32, "pqk")
                        for sub in range(2):
                            fo = fe + sub
                            half = ps[:, sub * SQ:(sub + 1) * SQ]
                            for j in range(NJ):
                                nc.tensor.matmul(half,
                                                 wq_t[j][:, :, fo * P:(fo + 1) * P],
                                                 y1T8[:, 2 * j:2 * j + 2, 0:SQ],
                                                 start=(j == 0),
                                                 stop=(not FL["bq"] and j == NJ - 1),
                                                 perf_mode=DR)
                            if FL["bq"]:
                                nc.tensor.matmul(half,
                                                 brow["bq"][:, fo * P:(fo + 1) * P],
                                                 ones[:], start=False, stop=True)
                        nc.vector.tensor_copy(
                            out=qT[:, fe:fe + 2, :],
                            in_=ps[:].rearrange("p (a b) -> p a b", b=SQ))

                    kT_chunk(0, 0)
                    kT_chunk(1, 0)
                    # stats (and their ACT sqrts) for tiles 4-7 queue before
                    # the first exp so the Sqrt/Exp tables load exactly once
                    mvrs = {tt: emit_ln1_stats(tt) for tt in range(NQ, NT)}
                    act_warm(ACTF.Exp)
                    # pair-0 scores for the first two kt-pairs start while the
                    # second half of the sequence is still loading/normalizing
                    for kp in (0, 1):
                        for h in (0, 1):
                            e01[h].append(score2(h, kp))
                    emit_ln1_apply(NQ, mvrs[NQ], ps_tp)
                    emit_ln1_apply(NQ + 1, mvrs[NQ + 1], ps_tp)
                    V_mm(0)
                    emit_ln1_apply(NQ + 2, mvrs[NQ + 2], ps_tp)
                    V_mm(1)
                    emit_ln1_apply(NQ + 3, mvrs[NQ + 3], ps_tp)
                    V_mm(2)
                    V_mm(3)

                kT_chunk(0, 1)
                kT_chunk(1, 1)
                for kt in range(4, NT):
                    V_mm(kt)
                for kp in (2, 3):
                    for h in (0, 1):
                        e01[h].append(score2(h, kp))
                kT_chunk(2, 0)
                kT_chunk(2, 1)

                with tc.tile_pool(name=pfx + "ps_ctx", bufs=2,
                                  space=bass.MemorySpace.PSUM) as ps_ctx:
                    def ctx_mm(pcs, h, kp, e):
                        for sub in range(2):
                            kt = 2 * kp + sub
                            nc.tensor.matmul(pcs[:], V[kt][:, h, :],
                                             e[:, sub * SQ:(sub + 1) * SQ],
                                             start=(kt == 0), stop=(kt == NT - 1),
                                             skip_group_check=True)

                    def ctx_norm(h, pcs):
                        fco, po = h // 2, (h % 2) * d
                        rb = _T(small_p, [d, SQ], F32, "rb")
                        nc.vector.reciprocal(rb[:], pcs[d:2 * d, :])
                        nc.vector.tensor_mul(ctxT8[po:po + 64, fco, :],
                                             pcs[0:d, :], rb[:])

                    pcs0 = {h: _T(ps_ctx, [P, SQ], F32, "pc") for h in (0, 1)}
                    for kp in range(NT // 2):
                        for h in (0, 1):
                            ctx_mm(pcs0[h], h, kp, e01[h][kp])
                    for h in (0, 1):
                        ctx_norm(h, pcs0[h])
                    del e01, pcs0

                    LAG = 2
                    for fc in range(1, NF):
                        pair = (2 * fc, 2 * fc + 1)
                        es = {h: [] for h in pair}
                        pcs = {h: _T(ps_ctx, [P, SQ], F32, "pc") for h in pair}
                        for kp in range(NT // 2):
                            for h in pair:
                                es[h].append(score2(h, kp))
                            if kp == 0 and fc + 2 < NF:
                                kT_chunk(fc + 2, 0)
                                kT_chunk(fc + 2, 1)
                            if kp >= LAG:
                                for h in pair:
                                    ctx_mm(pcs[h], h, kp - LAG, es[h][kp - LAG])
                        for kp in range(NT // 2 - LAG, NT // 2):
                            for h in pair:
                                ctx_mm(pcs[h], h, kp, es[h][kp])
                        for h in pair:
                            ctx_norm(h, pcs[h])
                    act_warm(ACTF.Sqrt)

            _p1.close()

        # ================= phase 2: Wo, LN2, FFN =================
        with (
            tc.tile_pool(name=pfx + "wo", bufs=2 * NJ) as wo_p,
            tc.tile_pool(name=pfx + "x2", bufs=NQ) as x2_p,
            tc.tile_pool(name=pfx + "y2", bufs=2) as y2_p,
            tc.tile_pool(name=pfx + "y2T", bufs=1) as y2T_p,
            tc.tile_pool(name=pfx + "x2t", bufs=2) as x2t_p,
        ):
            wo_t = [_T(wo_p, [P, 2, SQ], F8, "woW") for _ in range(2 * NJ)]
            for i in range(2 * NJ):
                nc.sync.dma_start(out=wo_t[i][:], in_=t["wo"][i])
            w1a_t = [_T(w1a_p, [P, 2, F], F8, "w1a") for _ in range(NA1 // 2)]
            for j in range(NA1 // 2):
                nc.sync.dma_start(out=w1a_t[j][:], in_=t["w1a"][j])
            w1_t = [_T(w1_p, [P, (NF - NA1) * P], BF16, "w1s")
                    for _ in range(NF1)]
            for f1 in range(NF1):
                nc.sync.dma_start(out=w1_t[f1][:], in_=t["w1b"][f1])
            w2a_t = [_T(wh_p, [P, 2, SQ], F8, "w2a")
                     for _ in range(2 * (NA2 // 2))]
            for i in range(2 * (NA2 // 2)):
                nc.sync.dma_start(out=w2a_t[i][:], in_=t["w2a"][i])
            w2_t = [_T(wh_p, [P, SQ], BF16, "wh")
                    for _ in range(2 * (NF1 - NA2))]
            for i in range(2 * (NF1 - NA2)):
                nc.sync.dma_start(out=w2_t[i][:], in_=t["w2b"][i])

            x2 = [_T(x2_p, [P, D], F32, "x2") for _ in range(NQ)]
            y2T8 = _T(y2T_p, [P, NA1, SQ], F8, "y2T8")
            y2T = _T(y2T_p, [P, NF - NA1, SQ], BF16, "y2T")

            with (
                tc.tile_pool(name=pfx + "ps_wo", bufs=2,
                             space=bass.MemorySpace.PSUM) as ps_wo,
                tc.tile_pool(name=pfx + "ps_tp2", bufs=2,
                             space=bass.MemorySpace.PSUM) as ps_tp2,
            ):
                def emit_wo(qt, ps):
                    for dh in range(2):
                        half = ps[:, dh * SQ:(dh + 1) * SQ]
                        for j in range(NJ):
                            nc.tensor.matmul(
                                half,
                                ctxT8[:, 2 * j:2 * j + 2, qt * P:(qt + 1) * P],
                                wo_t[dh * NJ + j][:],
                                start=(j == 0), stop=(j == NJ - 1),
                                perf_mode=DR)

                def emit_ln2(qt, ps):
                    xt2 = _T(x2t_p, [P, D], F32, "x2t")
                    nc.scalar.activation(out=xt2[:], in_=ps[:], func=ACTF.Copy,
                                         scale=1.0 / SCO)
                    nc.vector.tensor_add(x2[qt][:], xt2[:], xres[qt][:])
                    if FL["bo"]:
                        nc.vector.tensor_add(x2[qt][:], x2[qt][:], bo_b[:])
                    yt = _T(y2_p, [P, D], BF16, "y2")
                    mv, rs = _ln_stats(nc, small_p, x2[qt][:], epst)
                    nc.vector.tensor_scalar(out=yt[:], in0=x2[qt][:],
                                            scalar1=mv[:, 0:1], scalar2=rs[:],
                                            op0=ALU.subtract, op1=ALU.mult)
                    return yt

                def emit_y2T(qt, yt):
                    for fc in range(NF):
                        pt = _T(ps_tp2, [P, P], BF16, "pt2")
                        nc.tensor.transpose(pt[:], yt[:, fc * P:(fc + 1) * P],
                                            ident[:])
                        dst = (y2T8[:, fc, qt * P:(qt + 1) * P] if fc < NA1
                               else y2T[:, fc - NA1, qt * P:(qt + 1) * P])
                        nc.scalar.copy(out=dst, in_=pt[:])

                wops = [_T(ps_wo, [P, D], F32, "pwo") for _ in range(2)]
                emit_wo(0, wops[0])
                emit_wo(1, wops[1])
                y0 = emit_ln2(0, wops[0])
                y1_ = emit_ln2(1, wops[1])
                wops2 = [_T(ps_wo, [P, D], F32, "pwo") for _ in range(2)]
                emit_wo(2, wops2[0])
                emit_y2T(0, y0)
                emit_wo(3, wops2[1])
                emit_y2T(1, y1_)
                y2_ = emit_ln2(2, wops2[0])
                y3 = emit_ln2(3, wops2[1])
                emit_y2T(2, y2_)
                emit_y2T(3, y3)

            act_warm(ACTF.Gelu)
            # FFN1: h^T = gelu(W1' y2 + b1'), f1 pairs share a PSUM buf
            with (
                tc.tile_pool(name=pfx + "hT", bufs=1) as hT_p,
                tc.tile_pool(name=pfx + "xo", bufs=NQ) as xo_p,
                tc.tile_pool(name=pfx + "ps_f1", bufs=2,
                             space=bass.MemorySpace.PSUM) as ps_f1,
                tc.tile_pool(name=pfx + "ps_4", bufs=4,
                             space=bass.MemorySpace.PSUM) as ps_4,
            ):
                hT8 = _T(hT_p, [P, NA2, SQ], F8, "hT8")
                hT = _T(hT_p, [P, NF1 - NA2, SQ], BF16, "hT")
                for fe in range(0, NF1, 2):
                    ps = _T(ps_f1, [P, 2 * SQ], F32, "pf1")
                    for sub in range(2):
                        f1 = fe + sub
                        half = ps[:, sub * SQ:(sub + 1) * SQ]
                        for j in range(NA1 // 2):
                            nc.tensor.matmul(half,
                                             w1a_t[j][:, :, f1 * P:(f1 + 1) * P],
                                             y2T8[:, 2 * j:2 * j + 2, :],
                                             start=(j == 0), stop=False,
                                             perf_mode=DR)
                        nb = NF - NA1
                        for fi in range(nb):
                            nc.tensor.matmul(half,
                                             w1_t[f1][:, fi * P:(fi + 1) * P],
                                             y2T[:, fi, :],
                                             start=False,
                                             stop=(not FL["b1"] and fi == nb - 1))
                        if FL["b1"]:
                            nc.tensor.matmul(half,
                                             brow["b1"][:, f1 * P:(f1 + 1) * P],
                                             ones[:], start=False, stop=True)
                    out_sl = (hT8[:, fe:fe + 2, :] if fe < NA2
                              else hT[:, fe - NA2:fe - NA2 + 2, :])
                    nc.scalar.activation(out=out_sl.rearrange(
                        "p a b -> p (a b)"), in_=ps[:], func=ACTF.Gelu,
                        scale=1.0 / SC1)

                # FFN2 + residual: out = x2 + h @ W2 + b2
                xout = [_T(xo_p, [P, D], F32, "xo") for _ in range(NQ)]
                nb2 = NF1 - NA2
                for dh in range(2):
                    dsl = slice(dh * SQ, (dh + 1) * SQ)
                    ps4 = [_T(ps_4, [P, SQ], F32, "p4") for _ in range(NQ)]
                    for j in range(NA2 // 2):
                        for qt in range(NQ):
                            nc.tensor.matmul(ps4[qt][:],
                                             hT8[:, 2 * j:2 * j + 2,
                                                 qt * P:(qt + 1) * P],
                                             w2a_t[dh * (NA2 // 2) + j][:],
                                             start=(j == 0), stop=False,
                                             perf_mode=DR)
                    for f1 in range(nb2):
                        for qt in range(NQ):
                            nc.tensor.matmul(ps4[qt][:],
                                             hT[:, f1, qt * P:(qt + 1) * P],
                                             w2_t[dh * nb2 + f1][:],
                                             start=False,
                                             stop=(not FL["b2"] and f1 == nb2 - 1))
                    for qt in range(NQ):
                        if FL["b2"]:
                            nc.tensor.matmul(ps4[qt][:], ones[:, 0:P],
                                             brow["b2"][:, dsl],
                                             start=False, stop=True)
                        xsc = _T(x2t_p, [P, SQ], F32, "xsc")
                        nc.scalar.activation(out=xsc[:], in_=ps4[qt][:],
                                             func=ACTF.Copy, scale=1.0 / SC2)
                        nc.vector.tensor_add(xout[qt][:, dsl], xsc[:],
                                             x2[qt][:, dsl])
                        nc.sync.dma_start(out=out3[qt][:, dsl],
                                          in_=xout[qt][:, dsl])


_NC = {}
_ALL_FLAGS = ("bq", "bk", "bv", "bo", "b1", "b2")


def _get_nc(flags=None, reps=1):
    if flags is None:
        flags = {k: True for k in _ALL_FLAGS}
    key = (tuple(sorted(flags.items())), reps)
    if key not in _NC:
        _NC[key] = _build_program(dict(flags), reps=reps)
    return _NC[key]


def _q8(w, scale):
    return np.clip(w * scale, -224.0, 224.0).astype(E4)


def _prep_inputs(inputs):
    """Host-side folding + per-core shard maps."""
    x = np.asarray(inputs["x"], np.float32)
    mask = np.asarray(inputs["mask"], np.float32)
    g1 = np.asarray(inputs["ln1_g"], np.float32)
    b1n = np.asarray(inputs["ln1_b"], np.float32)
    g2 = np.asarray(inputs["ln2_g"], np.float32)
    b2n = np.asarray(inputs["ln2_b"], np.float32)
    Wq = np.asarray(inputs["Wq"], np.float32); bq = np.asarray(inputs["bq"], np.float32)
    Wk = np.asarray(inputs["Wk"], np.float32); bk = np.asarray(inputs["bk"], np.float32)
    Wv = np.asarray(inputs["Wv"], np.float32); bv = np.asarray(inputs["bv"], np.float32)
    Wo = np.asarray(inputs["Wo"], np.float32); bo = np.asarray(inputs["bo"], np.float32)
    W1 = np.asarray(inputs["W1"], np.float32); b1 = np.asarray(inputs["b1"], np.float32)
    W2 = np.asarray(inputs["W2"], np.float32); b2 = np.asarray(inputs["b2"], np.float32)

    scale = d ** -0.5
    # fold LN gains/biases (and q scale) into the projection weights
    Wq_e = (g1[:, None] * Wq) * scale
    bq_e = (b1n @ Wq + bq) * scale
    Wk_e = g1[:, None] * Wk
    bk_e = b1n @ Wk + bk
    Wv_e = g1[:, None] * Wv
    bv_e = b1n @ Wv + bv
    W1_e = g2[:, None] * W1
    b1_e = b2n @ W1 + b1

    def tile_dr(W, sc):
        # [Din, Dout] -> [NJ(j), P, 2(i), Dout] fp8, f_in = (2j+i)*128 + p
        return np.ascontiguousarray(
            _q8(W, sc).reshape(NJ, 2, P, D).transpose(0, 2, 1, 3))

    wq_h = tile_dr(Wq_e, SCQ)
    wk_h = tile_dr(Wk_e, SCK)
    wv_h = tile_dr(Wv_e, SCV)
    # Wo -> [2(dh)*NJ(j), P, 2(i), SQ] fp8
    wo_h = np.ascontiguousarray(
        _q8(Wo, SCO).reshape(NJ, 2, P, 2, SQ).transpose(3, 0, 2, 1, 4)
        .reshape(2 * NJ, P, 2, SQ))
    # FFN1: f_in chunks [0, NA1) in fp8 DoubleRow pairs, rest bf16 (both *SC1)
    ka = NA1 * P
    w1a_h = np.ascontiguousarray(
        _q8(W1_e[:ka], SC1).reshape(NA1 // 2, 2, P, F).transpose(0, 2, 1, 3))
    w1b_h = np.ascontiguousarray(
        (W1_e[ka:] * SC1).reshape(NF - NA1, P, NF1, P)
        .transpose(2, 1, 0, 3).reshape(NF1, P, (NF - NA1) * P)).astype(BF)
    # FFN2: f1 chunks [0, NA2) in fp8 DoubleRow pairs, rest bf16 (both *SC2)
    kb = NA2 * P
    w2a_h = np.ascontiguousarray(
        _q8(W2[:kb], SC2).reshape(NA2 // 2, 2, P, 2, SQ)
        .transpose(3, 0, 2, 1, 4).reshape(2 * (NA2 // 2), P, 2, SQ))
    w2b_h = np.ascontiguousarray(
        (W2[kb:] * SC2).reshape(NF1 - NA2, P, 2, SQ)
        .transpose(2, 0, 1, 3).reshape(2 * (NF1 - NA2), P, SQ)).astype(BF)

    flags = {
        "bq": bool(np.any(bq_e)), "bk": bool(np.any(bk_e)),
        "bv": bool(np.any(bv_e)), "bo": bool(np.any(bo)),
        "b1": bool(np.any(b1_e)), "b2": bool(np.any(b2)),
    }
    shared = {
        "ident": np.eye(P, dtype=BF),
        "wq": wq_h, "wk": wk_h, "wv": wv_h, "wo": wo_h,
        "w1a": w1a_h, "w1b": w1b_h, "w2a": w2a_h, "w2b": w2b_h,
        "bq": (bq_e * SCQ).reshape(1, D).astype(BF),
        "bk": (bk_e * SCK).reshape(1, D).astype(BF),
        "bv": (bv_e * SCV).reshape(1, D).astype(BF),
        "bo": bo.reshape(1, D).astype(np.float32),
        "b1": (b1_e * SC1).reshape(1, F).astype(BF),
        "b2": (b2 * SC2).reshape(1, D).astype(BF),
    }

    in_maps = []
    for c in range(8):
        b, hf = c // 2, c % 2
        x_rot = np.ascontiguousarray(np.roll(x[b], -SQ * hf, axis=0))
        mq = mask[b, 0, SQ * hf:SQ * (hf + 1), :]          # [512 q, 1024 k]
        mT = np.roll(mq.T, -SQ * hf, axis=0)               # [1024 k, 512 q]
        # kt-pair layout: [4, 128, 1024] with pair tiles side by side
        mT2 = np.ascontiguousarray(
            mT.reshape(NT // 2, 2, P, SQ).transpose(0, 2, 1, 3)
            .reshape(NT // 2, P, 2 * SQ)).astype(np.uint8)
        m = dict(shared)
        m["x"] = np.ascontiguousarray(x_rot[:SQ])
        m["xk"] = np.ascontiguousarray(x_rot[SQ:]).astype(BF)
        m["maskT2"] = mT2
        in_maps.append(m)
    return in_maps, flags


def run(inputs, trace=False, **kw):
    in_maps, flags = _prep_inputs(inputs)
    nc = _get_nc(flags)
    res = run_bass_kernel_spmd(nc, in_maps, core_ids=list(range(8)),
                               trace=trace, **kw)
    out = np.empty((B, S, D), np.float32)
    for c in range(8):
        b, hf = c // 2, c % 2
        out[b, SQ * hf:SQ * (hf + 1), :] = res.results[c]["out"]
    return out, res


def kernel(**inputs) -> np.ndarray:
    out, _ = run(inputs, trace=False)
    return out


# revision 31
# speedup vs baseline: 2.5868x; 1.4718x over previous
"""Trainium2 Bass kernel for a dense transformer block (B=4,S=1024,D=1024,F=4096,H=16).

Sharding: 8 cores = (batch b in 0..3) x (seq half). Pure SPMD, no collectives:
the host rotates each core's tokens so its 512 query rows are always rows
0..511 of the rotated sequence; K/V cover the full (rotated) sequence.

v2: fp8e4m3 DoubleRow matmuls (0.5 cyc/row) for the QKV projections and Wo.
  - y1 = LN1(x) transposed to a single interleaved fp8 tile y1T8 [128, 8, 1024]
    so each DoubleRow matmul contracts 256 features (2 chunks of 128).
  - weights are pre-scaled on host (Wq*128, Wk*16, Wv*16, Wo*16) to lift the
    0.02-scale entries out of fp8 subnormal range; the score exp absorbs
    1/(128*16) via its activation scale, the softmax ones-column is set to 16
    to absorb V's scale, and an ACT copy with scale=1/16 rescales Wo's output.
  - scores/ctx and the FFN stay bf16 (fp8 there costs too much accuracy).
  - FFN additionally runs a mixed-precision contraction: the first NA1=4 (of
    8) f_in chunks of FFN1 and NA2=2 (of 32) f1 chunks of FFN2 go through fp8
    DoubleRow, the rest stays bf16 (both halves carry the 16x weight scale so
    they share one PSUM accumulation; gelu's activation scale and an ACT copy
    undo it). Measured rel err on hw: 1.5e-2 (gate 2e-2).
Scheduling: x DMAs lead the SP queue (weights follow, FFN weights prefetched
through outer-scope stream pools; x tiles 4-7 ship as bf16 since they only
feed LN1 -> fp8), LN runs split 0-3 / 4-7 so qT and the kT half-chunks start
as soon as their token tiles are normalized, exps are batched over kt-pairs
([128,1024] PSUM reads), dependency-free dummy activations preload the ACT
Sqrt/Exp/Gelu tables off the critical path, and V-ones memsets + small DMAs
ride GpSimd. The softmax row-sum rides the ctx matmul as 16.0-columns of
V_aug; the per-(b,h) additive attn_bias is constant over q and k so softmax
cancels it (no-op).
"""

import numpy as np
import ml_dtypes

import concourse.bass as bass
import concourse.mybir as mybir
import concourse.tile as tile
from concourse import bacc
from concourse.bass_utils import run_bass_kernel_spmd

F32 = mybir.dt.float32
BF16 = mybir.dt.bfloat16
F8 = mybir.dt.float8e4
BF = ml_dtypes.bfloat16
E4 = ml_dtypes.float8_e4m3

B, S, D, F, H = 4, 1024, 1024, 4096, 16
d = D // H          # 64
P = 128             # partitions
SQ = 512            # queries per core
EPS = 1e-5
NT = S // P         # 8 token tiles (full seq)
NQ = SQ // P        # 4 query tiles
NF = D // P         # 8 feature chunks
NJ = NF // 2        # 4 fp8 contraction pairs
NF1 = F // P        # 32 ffn chunks

SCQ = 128.0         # host pre-scale on Wq (includes d^-0.5 fold)
SCK = 16.0          # host pre-scale on Wk
SCV = 16.0          # host pre-scale on Wv (absorbed by 16.0 ones-column)
SCO = 16.0          # host pre-scale on Wo (rescaled by ACT copy 1/16)
SC1 = 16.0          # pre-scale on W1 (both halves; gelu scale absorbs 1/16)
SC2 = 16.0          # pre-scale on W2 (both halves; ACT copy absorbs 1/16)
NA1 = 4             # f_in chunks of FFN1 run in fp8 (alpha = NA1/NF)
NA2 = 2             # f1 chunks of FFN2 run in fp8 (alpha = NA2/NF1)

AX = mybir.AxisListType
ALU = mybir.AluOpType
ACTF = mybir.ActivationFunctionType
DR = mybir.MatmulPerfMode.DoubleRow


def _T(pool, shape, dtype, tag):
    return pool.tile(shape, dtype, name=tag, tag=tag)


def _pbcast(ap, p):
    """Partition-broadcast a [1, N] DRAM AP to [p, N]."""
    return bass.AP(tensor=ap.tensor, offset=ap.offset, ap=[[0, p]] + list(ap.ap[1:]))


def _build_program(FL, reps=1):
    nc = bacc.Bacc("TRN2", target_bir_lowering=False, debug=False)

    t = {}
    t["x"] = nc.dram_tensor("x", [SQ, D], F32, kind="ExternalInput").ap()
    t["xk"] = nc.dram_tensor("xk", [SQ, D], BF16, kind="ExternalInput").ap()
    t["maskT2"] = nc.dram_tensor("maskT2", [P, NT // 2, 2 * SQ], BF16,
                                 kind="ExternalInput").ap()
    t["wq"] = nc.dram_tensor("wq", [P, NJ, 2, D], F8, kind="ExternalInput").ap()
    t["wk"] = nc.dram_tensor("wk", [P, NJ, 2, D], F8, kind="ExternalInput").ap()
    t["wv"] = nc.dram_tensor("wv", [P, NJ, 2, D], F8, kind="ExternalInput").ap()
    t["wo"] = nc.dram_tensor("wo", [P, 2 * NJ, 2, SQ], F8,
                             kind="ExternalInput").ap()
    t["w1a"] = nc.dram_tensor("w1a", [P, NA1 // 2, 2, F], F8,
                              kind="ExternalInput").ap()
    t["w1b"] = nc.dram_tensor("w1b", [NF1 // 8, P, 8, (NF - NA1) * P], BF16,
                              kind="ExternalInput").ap()
    t["w2a"] = nc.dram_tensor("w2a", [P, 2 * (NA2 // 2), 2, SQ], F8,
                              kind="ExternalInput").ap()
    t["w2b"] = nc.dram_tensor("w2b", [2 * (NF1 - NA2) // 5, P, 5, SQ], BF16,
                              kind="ExternalInput").ap()
    t["bq"] = nc.dram_tensor("bq", [1, D], BF16, kind="ExternalInput").ap()
    t["bk"] = nc.dram_tensor("bk", [1, D], BF16, kind="ExternalInput").ap()
    t["bv"] = nc.dram_tensor("bv", [1, D], BF16, kind="ExternalInput").ap()
    t["bo"] = nc.dram_tensor("bo", [1, D], F32, kind="ExternalInput").ap()
    t["b1"] = nc.dram_tensor("b1", [1, F], BF16, kind="ExternalInput").ap()
    t["b2"] = nc.dram_tensor("b2", [1, D], BF16, kind="ExternalInput").ap()
    t["ident"] = nc.dram_tensor("ident", [P, P], BF16, kind="ExternalInput").ap()
    t["out"] = nc.dram_tensor("out", [SQ, D], F32, kind="ExternalOutput").ap()

    with tile.TileContext(nc) as tc:
        for rep in range(reps):
            _trace(nc, tc, t, FL, pfx=f"r{rep}_" if reps > 1 else "")
    nc.compile()
    return nc


def _ln_stats(nc, pool, x_ap, epst):
    """Return [P,1] mean and rstd tiles for a [P, 1024] input."""
    st = _T(pool, [P, 2, 6], F32, "st")
    xr = x_ap.rearrange("p (a b) -> p a b", b=512)
    for sg in range(2):
        nc.vector.bn_stats(out=st[:, sg, :], in_=xr[:, sg, :])
    mv = _T(pool, [P, 2], F32, "mv")
    nc.vector.bn_aggr(out=mv[:], in_=st[:])
    rs = _T(pool, [P, 1], F32, "rs")
    nc.scalar.activation(out=rs[:], in_=mv[:, 1:2], func=ACTF.Sqrt,
                         bias=epst[:], scale=1.0)
    nc.vector.reciprocal(out=rs[:], in_=rs[:])
    return mv, rs


def _trace(nc, tc, t, FL, pfx=""):
    x3 = t["x"].rearrange("(n p) c -> n p c", p=P)
    xk3 = t["xk"].rearrange("(n p) c -> n p c", p=P)
    out3 = t["out"].rearrange("(n p) c -> n p c", p=P)

    with (
        tc.tile_pool(name=pfx + "const", bufs=1) as const_p,
        tc.tile_pool(name=pfx + "small", bufs=4) as small_p,
        tc.tile_pool(name=pfx + "xres", bufs=NQ) as xres_p,
        tc.tile_pool(name=pfx + "ctxT", bufs=1) as ctxT_p,
        tc.tile_pool(name=pfx + "w1a", bufs=1) as w1a_p,
        tc.tile_pool(name=pfx + "w1s", bufs=2) as w1_p,
        tc.tile_pool(name=pfx + "w2s", bufs=3) as wh_p,
        tc.tile_pool(name=pfx + "w2a", bufs=1) as w2a_p,
    ):
        # ---- constants ----
        ident = _T(const_p, [P, P], BF16, "ident")
        nc.gpsimd.dma_start(out=ident[:], in_=t["ident"])
        if any(FL[k] for k in ("bq", "bk", "bv", "b1", "b2")):
            ones = _T(const_p, [1, SQ], BF16, "ones")
            nc.vector.memset(ones[:], 1.0)
        epst = _T(const_p, [P, 1], F32, "epst")
        nc.vector.memset(epst[:], EPS)
        actws = _T(const_p, [P, 1], F32, "actws")

        def act_warm(func):
            # dependency-free dummy that hides the 1.3us ACT table load
            nc.scalar.activation(out=actws[:], in_=epst[:], func=func)
        if FL["bo"]:
            bo_b = _T(const_p, [P, D], F32, "bo_b")
            nc.sync.dma_start(out=bo_b[:], in_=_pbcast(t["bo"], P))
        brow = {}
        for nm in ("bq", "bk", "bv", "b1", "b2"):
            if not FL[nm]:
                continue
            sz = F if nm == "b1" else D
            brow[nm] = _T(const_p, [1, sz], BF16, f"brow_{nm}")
            nc.sync.dma_start(out=brow[nm][:], in_=t[nm])

        xres = [_T(xres_p, [P, D], F32, "xres") for _ in range(NQ)]
        ctxT8 = _T(ctxT_p, [P, NF, SQ], F8, "ctxT8")

        # ================= phase 1: LN1, y1T8, QKV, attention =================
        with (
            tc.tile_pool(name=pfx + "y1T8", bufs=1) as y1T8_p,
            tc.tile_pool(name=pfx + "qkT", bufs=1) as qkT_p,
            tc.tile_pool(name=pfx + "wqk", bufs=1) as wqk_p,
            tc.tile_pool(name=pfx + "wv", bufs=1) as wv_p,
            tc.tile_pool(name=pfx + "Vp", bufs=NT) as V_p,
            tc.tile_pool(name=pfx + "mk", bufs=1) as mk_p,
            tc.tile_pool(name=pfx + "es", bufs=10) as e_p,
            tc.tile_pool(name=pfx + "xs", bufs=4) as xs_p,
            tc.tile_pool(name=pfx + "y1", bufs=3) as y1_p,
        ):
            y1T8 = _T(y1T8_p, [P, NF, S], F8, "y1T8")
            qT = _T(qkT_p, [P, NF, SQ], BF16, "qT")
            kT = _T(qkT_p, [P, NF, S], BF16, "kT")

            # SP DMA queue order: x0-3, wq, xk4-7, wk, m, wv, wo, w1, w2.
            # Tiles 4-7 only feed LN1 -> fp8, so the host ships them bf16.
            xt_all = []
            for tt in range(NT):
                xt_all.append(xres[tt] if tt < NQ
                              else _T(xs_p, [P, D], BF16, "xs"))
            act_warm(ACTF.Sqrt)
            nc.sync.dma_start(out=xt_all[0][:, 0:SQ], in_=x3[0][:, 0:SQ])
            nc.sync.dma_start(out=xt_all[0][:, SQ:D], in_=x3[0][:, SQ:D])
            for tt in range(1, NQ):
                nc.sync.dma_start(out=xt_all[tt][:], in_=x3[tt])
            wq_a = _T(wqk_p, [P, NJ, 2, D], F8, "wq")
            nc.sync.dma_start(out=wq_a[:], in_=t["wq"])
            wq_t = [wq_a[:, j, :, :] for j in range(NJ)]
            for tt in range(NQ, NT):
                nc.sync.dma_start(out=xt_all[tt][:], in_=xk3[tt - NQ])
            wk_a = _T(wqk_p, [P, NJ, 2, D], F8, "wk")
            nc.sync.dma_start(out=wk_a[:], in_=t["wk"])
            wk_t = [wk_a[:, j, :, :] for j in range(NJ)]
            m2a = _T(mk_p, [P, NT // 2, 2 * SQ], BF16, "m2")
            nc.sync.dma_start(out=m2a[:], in_=t["maskT2"])
            m2 = [m2a[:, kp, :] for kp in range(NT // 2)]
            wv_a = _T(wv_p, [P, NJ, 2, D], F8, "wv")
            nc.sync.dma_start(out=wv_a[:], in_=t["wv"])
            wv_t = [wv_a[:, j, :, :] for j in range(NJ)]

            def emit_ln1_stats(tt):
                return _ln_stats(nc, small_p, xt_all[tt][:], epst)

            def emit_ln1_apply(tt, mvrs, ps_tp):
                xt = xt_all[tt]
                mv, rs = mvrs
                yt = _T(y1_p, [P, D], BF16, "y1")
                nc.vector.tensor_scalar(out=yt[:], in0=xt[:],
                                        scalar1=mv[:, 0:1], scalar2=rs[:],
                                        op0=ALU.subtract, op1=ALU.mult)
                for fc in range(NF):
                    pt = _T(ps_tp, [P, P], BF16, "pt")
                    nc.tensor.transpose(pt[:], yt[:, fc * P:(fc + 1) * P],
                                        ident[:])
                    dst = y1T8[:, fc, tt * P:(tt + 1) * P]
                    if fc < 6:
                        nc.scalar.copy(out=dst, in_=pt[:])
                    else:
                        nc.vector.tensor_copy(out=dst, in_=pt[:])

            from contextlib import ExitStack
            _p1 = ExitStack()
            ps_s2 = _p1.enter_context(tc.tile_pool(
                name=pfx + "ps_s2", bufs=2, space=bass.MemorySpace.PSUM))
            ps_qkv = _p1.enter_context(tc.tile_pool(
                name=pfx + "ps_qkv", bufs=1, space=bass.MemorySpace.PSUM))
            if True:
                def kT_chunk(fo, th):
                    tsl = slice(th * SQ, (th + 1) * SQ)
                    ps = _T(ps_qkv, [P, 2 * SQ], F32, "pqk")
                    half = ps[:, 0:SQ] if th == 0 else ps[:, SQ:2 * SQ]
                    for j in range(NJ):
                        nc.tensor.matmul(half,
                                         wk_t[j][:, :, fo * P:(fo + 1) * P],
                                         y1T8[:, 2 * j:2 * j + 2, tsl],
                                         start=(j == 0),
                                         stop=(not FL["bk"] and j == NJ - 1),
                                         perf_mode=DR)
                    if FL["bk"]:
                        nc.tensor.matmul(half,
                                         brow["bk"][:, fo * P:(fo + 1) * P],
                                         ones[:], start=False, stop=True)
                    nc.vector.tensor_copy(out=kT[:, fo, tsl], in_=half)

                def score2(h, kp):
                    """scores^T -> exp for one (head, kt-pair): E [128,1024]
                    covering key tiles 2kp, 2kp+1."""
                    fc, po = h // 2, (h % 2) * d
                    ps2 = _T(ps_s2, [P, 2 * SQ], F32, "ps2")
                    for sub in range(2):
                        kt = 2 * kp + sub
                        nc.tensor.matmul(ps2[:, sub * SQ:(sub + 1) * SQ],
                                         kT[po:po + d, fc, kt * P:(kt + 1) * P],
                                         qT[po:po + d, fc, :],
                                         start=True, stop=True)
                    e = _T(e_p, [P, 2 * SQ], BF16, "e")
                    nc.scalar.activation(out=e[:], in_=ps2[:], func=ACTF.Exp,
                                         scale=1.0 / (SCQ * SCK))
                    nc.vector.tensor_mul(e[:], e[:], m2[kp][:])
                    return e

                V = [_T(V_p, [P, H, 2 * d], BF16, "V") for _ in range(NT)]

                def V_mm(kt):
                    # V token-major, heads interleaved d v-cols then d 16-cols
                    nc.gpsimd.memset(V[kt][:, :, d:], SCV)
                    ps = _T(ps_s2, [P, 2 * SQ], F32, "ps2")
                    for fh in range(2):
                        half = ps[:, fh * SQ:(fh + 1) * SQ]
                        for j in range(NJ):
                            nc.tensor.matmul(half,
                                             y1T8[:, 2 * j:2 * j + 2,
                                                  kt * P:(kt + 1) * P],
                                             wv_t[j][:, :, fh * SQ:(fh + 1) * SQ],
                                             start=(j == 0),
                                             stop=(not FL["bv"] and j == NJ - 1),
                                             perf_mode=DR)
                        if FL["bv"]:
                            nc.tensor.matmul(half, ones[:, 0:P],
                                             brow["bv"][:, fh * SQ:(fh + 1) * SQ],
                                             start=False, stop=True)
                    nc.vector.tensor_copy(
                        out=V[kt][:, :, 0:d],
                        in_=ps[:].rearrange("p (a b) -> p a b", b=d))

                e01 = {0: [], 1: []}
                with tc.tile_pool(name=pfx + "ps_tp", bufs=2,
                                  space=bass.MemorySpace.PSUM) as ps_tp:
                    for tt in range(NQ):
                        emit_ln1_apply(tt, emit_ln1_stats(tt), ps_tp)

                    # q^T: fp8 DoubleRow, fo pairs share a PSUM buf (own 512 q)
                    for fe in range(0, NF, 2):
                        ps = _T(ps_qkv, [P, 2 * SQ], F32, "pqk")
                        for sub in range(2):
                            fo = fe + sub
                            half = ps[:, sub * SQ:(sub + 1) * SQ]
                            for j in range(NJ):
                                nc.tensor.matmul(half,
                                                 wq_t[j][:, :, fo * P:(fo + 1) * P],
                                                 y1T8[:, 2 * j:2 * j + 2, 0:SQ],
                                                 start=(j == 0),
                                                 stop=(not FL["bq"] and j == NJ - 1),
                                                 perf_mode=DR)
                            if FL["bq"]:
                                nc.tensor.matmul(half,
                                                 brow["bq"][:, fo * P:(fo + 1) * P],
                                                 ones[:], start=False, stop=True)
                        nc.vector.tensor_copy(
                            out=qT[:, fe:fe + 2, :],
                            in_=ps[:].rearrange("p (a b) -> p a b", b=SQ))

                    kT_chunk(0, 0)
                    kT_chunk(1, 0)
                    # stats (and their ACT sqrts) for tiles 4-7 queue before
                    # the first exp so the Sqrt/Exp tables load exactly once
                    mvrs = {tt: emit_ln1_stats(tt) for tt in range(NQ, NT)}
                    act_warm(ACTF.Exp)
                    # pair-0 scores for the first two kt-pairs start while the
                    # second half of the sequence is still loading/normalizing
                    for kp in (0, 1):
                        for h in (0, 1):
                            e01[h].append(score2(h, kp))
                    emit_ln1_apply(NQ, mvrs[NQ], ps_tp)
                    emit_ln1_apply(NQ + 1, mvrs[NQ + 1], ps_tp)
                    V_mm(0)
                    emit_ln1_apply(NQ + 2, mvrs[NQ + 2], ps_tp)
                    V_mm(1)
                    emit_ln1_apply(NQ + 3, mvrs[NQ + 3], ps_tp)
                    V_mm(2)
                    V_mm(3)

                kT_chunk(0, 1)
                kT_chunk(1, 1)
                for kt in range(4, NT):
                    V_mm(kt)
                for kp in (2, 3):
                    for h in (0, 1):
                        e01[h].append(score2(h, kp))
                kT_chunk(2, 0)
                kT_chunk(2, 1)

                with tc.tile_pool(name=pfx + "ps_ctx", bufs=2,
                                  space=bass.MemorySpace.PSUM) as ps_ctx:
                    def ctx_mm(pcs, h, kp, e):
                        for sub in range(2):
                            kt = 2 * kp + sub
                            nc.tensor.matmul(pcs[:], V[kt][:, h, :],
                                             e[:, sub * SQ:(sub + 1) * SQ],
                                             start=(kt == 0), stop=(kt == NT - 1),
                                             skip_group_check=True)

                    def ctx_norm(h, pcs):
                        fco, po = h // 2, (h % 2) * d
                        rb = _T(small_p, [d, SQ], F32, "rb")
                        nc.vector.reciprocal(rb[:], pcs[d:2 * d, :])
                        nc.vector.tensor_mul(ctxT8[po:po + 64, fco, :],
                                             pcs[0:d, :], rb[:])

                    pcs0 = {h: _T(ps_ctx, [P, SQ], F32, "pc") for h in (0, 1)}
                    for kp in range(NT // 2):
                        for h in (0, 1):
                            ctx_mm(pcs0[h], h, kp, e01[h][kp])
                    for h in (0, 1):
                        ctx_norm(h, pcs0[h])
                    del e01, pcs0

                    LAG = 2
                    for fc in range(1, NF):
                        pair = (2 * fc, 2 * fc + 1)
                        es = {h: [] for h in pair}
                        pcs = {h: _T(ps_ctx, [P, SQ], F32, "pc") for h in pair}
                        for kp in range(NT // 2):
                            for h in pair:
                                es[h].append(score2(h, kp))
                            if kp == 0 and fc + 2 < NF:
                                kT_chunk(fc + 2, 0)
                                kT_chunk(fc + 2, 1)
                            if kp >= LAG:
                                for h in pair:
                                    ctx_mm(pcs[h], h, kp - LAG, es[h][kp - LAG])
                        for kp in range(NT // 2 - LAG, NT // 2):
                            for h in pair:
                                ctx_mm(pcs[h], h, kp, es[h][kp])
                        for h in pair:
                            ctx_norm(h, pcs[h])
                    act_warm(ACTF.Sqrt)

            _p1.close()

        # ================= phase 2: Wo, LN2, FFN =================
        with (
            tc.tile_pool(name=pfx + "wo", bufs=1) as wo_p,
            tc.tile_pool(name=pfx + "x2", bufs=NQ) as x2_p,
            tc.tile_pool(name=pfx + "y2", bufs=2) as y2_p,
            tc.tile_pool(name=pfx + "y2T", bufs=1) as y2T_p,
            tc.tile_pool(name=pfx + "x2t", bufs=2) as x2t_p,
        ):
            wo_a = _T(wo_p, [P, 2 * NJ, 2, SQ], F8, "woW")
            nc.sync.dma_start(out=wo_a[:], in_=t["wo"])
            wo_t = [wo_a[:, i, :, :] for i in range(2 * NJ)]
            w1a_a = _T(w1a_p, [P, NA1 // 2, 2, F], F8, "w1a")
            nc.sync.dma_start(out=w1a_a[:], in_=t["w1a"])
            w1a_t = [w1a_a[:, j, :, :] for j in range(NA1 // 2)]
            w1g = [_T(w1_p, [P, 8, (NF - NA1) * P], BF16, "w1s")
                   for _ in range(NF1 // 8)]
            for g in range(NF1 // 8):
                nc.gpsimd.dma_start(out=w1g[g][:], in_=t["w1b"][g])
            w1_t = [w1g[f1 // 8][:, f1 % 8, :] for f1 in range(NF1)]
            w2a_a = _T(w2a_p, [P, 2 * (NA2 // 2), 2, SQ], F8, "w2a")
            nc.sync.dma_start(out=w2a_a[:], in_=t["w2a"])
            w2a_t = [w2a_a[:, i, :, :] for i in range(2 * (NA2 // 2))]
            ng2 = 2 * (NF1 - NA2) // 5
            w2g = [_T(wh_p, [P, 5, SQ], BF16, "wh") for _ in range(ng2)]
            for g in range(ng2):
                nc.gpsimd.dma_start(out=w2g[g][:], in_=t["w2b"][g])
            w2_t = [w2g[i // 5][:, i % 5, :] for i in range(2 * (NF1 - NA2))]

            x2 = [_T(x2_p, [P, D], F32, "x2") for _ in range(NQ)]
            y2T8 = _T(y2T_p, [P, NA1, SQ], F8, "y2T8")
            y2T = _T(y2T_p, [P, NF - NA1, SQ], BF16, "y2T")

            with (
                tc.tile_pool(name=pfx + "ps_wo", bufs=2,
                             space=bass.MemorySpace.PSUM) as ps_wo,
                tc.tile_pool(name=pfx + "ps_tp2", bufs=2,
                             space=bass.MemorySpace.PSUM) as ps_tp2,
            ):
                def emit_wo(qt, ps):
                    for dh in range(2):
                        half = ps[:, dh * SQ:(dh + 1) * SQ]
                        for j in range(NJ):
                            nc.tensor.matmul(
                                half,
                                ctxT8[:, 2 * j:2 * j + 2, qt * P:(qt + 1) * P],
                                wo_t[dh * NJ + j],
                                start=(j == 0), stop=(j == NJ - 1),
                                perf_mode=DR)

                def emit_ln2(qt, ps):
                    xt2 = _T(x2t_p, [P, D], F32, "x2t")
                    nc.scalar.activation(out=xt2[:], in_=ps[:], func=ACTF.Copy,
                                         scale=1.0 / SCO)
                    nc.vector.tensor_add(x2[qt][:], xt2[:], xres[qt][:])
                    if FL["bo"]:
                        nc.vector.tensor_add(x2[qt][:], x2[qt][:], bo_b[:])
                    yt = _T(y2_p, [P, D], BF16, "y2")
                    mv, rs = _ln_stats(nc, small_p, x2[qt][:], epst)
                    nc.vector.tensor_scalar(out=yt[:], in0=x2[qt][:],
                                            scalar1=mv[:, 0:1], scalar2=rs[:],
                                            op0=ALU.subtract, op1=ALU.mult)
                    return yt

                def emit_y2T(qt, yt):
                    for fc in range(NF):
                        pt = _T(ps_tp2, [P, P], BF16, "pt2")
                        nc.tensor.transpose(pt[:], yt[:, fc * P:(fc + 1) * P],
                                            ident[:])
                        dst = (y2T8[:, fc, qt * P:(qt + 1) * P] if fc < NA1
                               else y2T[:, fc - NA1, qt * P:(qt + 1) * P])
                        nc.scalar.copy(out=dst, in_=pt[:])

                wops = [_T(ps_wo, [P, D], F32, "pwo") for _ in range(2)]
                emit_wo(0, wops[0])
                emit_wo(1, wops[1])
                y0 = emit_ln2(0, wops[0])
                y1_ = emit_ln2(1, wops[1])
                wops2 = [_T(ps_wo, [P, D], F32, "pwo") for _ in range(2)]
                emit_wo(2, wops2[0])
                emit_y2T(0, y0)
                emit_wo(3, wops2[1])
                emit_y2T(1, y1_)
                y2_ = emit_ln2(2, wops2[0])
                y3 = emit_ln2(3, wops2[1])
                emit_y2T(2, y2_)
                emit_y2T(3, y3)

            act_warm(ACTF.Gelu)
            # FFN1: h^T = gelu(W1' y2 + b1'), f1 pairs share a PSUM buf
            with (
                tc.tile_pool(name=pfx + "hT", bufs=1) as hT_p,
                tc.tile_pool(name=pfx + "xo", bufs=NQ) as xo_p,
                tc.tile_pool(name=pfx + "ps_f1", bufs=2,
                             space=bass.MemorySpace.PSUM) as ps_f1,
                tc.tile_pool(name=pfx + "ps_4", bufs=4,
                             space=bass.MemorySpace.PSUM) as ps_4,
            ):
                hT8 = _T(hT_p, [P, NA2, SQ], F8, "hT8")
                hT = _T(hT_p, [P, NF1 - NA2, SQ], BF16, "hT")
                for fe in range(0, NF1, 2):
                    ps = _T(ps_f1, [P, 2 * SQ], F32, "pf1")
                    for sub in range(2):
                        f1 = fe + sub
                        half = ps[:, sub * SQ:(sub + 1) * SQ]
                        for j in range(NA1 // 2):
                            nc.tensor.matmul(half,
                                             w1a_t[j][:, :, f1 * P:(f1 + 1) * P],
                                             y2T8[:, 2 * j:2 * j + 2, :],
                                             start=(j == 0), stop=False,
                                             perf_mode=DR)
                        nb = NF - NA1
                        for fi in range(nb):
                            nc.tensor.matmul(half,
                                             w1_t[f1][:, fi * P:(fi + 1) * P],
                                             y2T[:, fi, :],
                                             start=False,
                                             stop=(not FL["b1"] and fi == nb - 1))
                        if FL["b1"]:
                            nc.tensor.matmul(half,
                                             brow["b1"][:, f1 * P:(f1 + 1) * P],
                                             ones[:], start=False, stop=True)
                    out_sl = (hT8[:, fe:fe + 2, :] if fe < NA2
                              else hT[:, fe - NA2:fe - NA2 + 2, :])
                    nc.scalar.activation(out=out_sl.rearrange(
                        "p a b -> p (a b)"), in_=ps[:], func=ACTF.Gelu,
                        scale=1.0 / SC1)

                # FFN2 + residual: out = x2 + h @ W2 + b2
                xout = [_T(xo_p, [P, D], F32, "xo") for _ in range(NQ)]
                nb2 = NF1 - NA2
                for dh in range(2):
                    dsl = slice(dh * SQ, (dh + 1) * SQ)
                    ps4 = [_T(ps_4, [P, SQ], F32, "p4") for _ in range(NQ)]
                    for j in range(NA2 // 2):
                        for qt in range(NQ):
                            nc.tensor.matmul(ps4[qt][:],
                                             hT8[:, 2 * j:2 * j + 2,
                                                 qt * P:(qt + 1) * P],
                                             w2a_t[dh * (NA2 // 2) + j],
                                             start=(j == 0), stop=False,
                                             perf_mode=DR)
                    for f1 in range(nb2):
                        for qt in range(NQ):
                            nc.tensor.matmul(ps4[qt][:],
                                             hT[:, f1, qt * P:(qt + 1) * P],
                                             w2_t[dh * nb2 + f1],
                                             start=False,
                                             stop=(not FL["b2"] and f1 == nb2 - 1))
                    for qt in range(NQ):
                        if FL["b2"]:
                            nc.tensor.matmul(ps4[qt][:], ones[:, 0:P],
                                             brow["b2"][:, dsl],
                                             start=False, stop=True)
                        xsc = _T(x2t_p, [P, SQ], F32, "xsc")
                        nc.scalar.activation(out=xsc[:], in_=ps4[qt][:],
                                             func=ACTF.Copy, scale=1.0 / SC2)
                        nc.vector.tensor_add(xout[qt][:, dsl], xsc[:],
                                             x2[qt][:, dsl])
                        nc.sync.dma_start(out=out3[qt][:, dsl],
                                          in_=xout[qt][:, dsl])


_NC = {}
_ALL_FLAGS = ("bq", "bk", "bv", "bo", "b1", "b2")


def _get_nc(flags=None, reps=1):
    if flags is None:
        flags = {k: True for k in _ALL_FLAGS}
    key = (tuple(sorted(flags.items())), reps)
    if key not in _NC:
        _NC[key] = _build_program(dict(flags), reps=reps)
    return _NC[key]


def _q8(w, scale):
    return np.clip(w * scale, -224.0, 224.0).astype(E4)


def _prep_inputs(inputs):
    """Host-side folding + per-core shard maps."""
    x = np.asarray(inputs["x"], np.float32)
    mask = np.asarray(inputs["mask"], np.float32)
    g1 = np.asarray(inputs["ln1_g"], np.float32)
    b1n = np.asarray(inputs["ln1_b"], np.float32)
    g2 = np.asarray(inputs["ln2_g"], np.float32)
    b2n = np.asarray(inputs["ln2_b"], np.float32)
    Wq = np.asarray(inputs["Wq"], np.float32); bq = np.asarray(inputs["bq"], np.float32)
    Wk = np.asarray(inputs["Wk"], np.float32); bk = np.asarray(inputs["bk"], np.float32)
    Wv = np.asarray(inputs["Wv"], np.float32); bv = np.asarray(inputs["bv"], np.float32)
    Wo = np.asarray(inputs["Wo"], np.float32); bo = np.asarray(inputs["bo"], np.float32)
    W1 = np.asarray(inputs["W1"], np.float32); b1 = np.asarray(inputs["b1"], np.float32)
    W2 = np.asarray(inputs["W2"], np.float32); b2 = np.asarray(inputs["b2"], np.float32)

    scale = d ** -0.5
    # fold LN gains/biases (and q scale) into the projection weights
    Wq_e = (g1[:, None] * Wq) * scale
    bq_e = (b1n @ Wq + bq) * scale
    Wk_e = g1[:, None] * Wk
    bk_e = b1n @ Wk + bk
    Wv_e = g1[:, None] * Wv
    bv_e = b1n @ Wv + bv
    W1_e = g2[:, None] * W1
    b1_e = b2n @ W1 + b1

    def tile_dr(W, sc):
        # [Din, Dout] -> [NJ(j), P, 2(i), Dout] fp8, f_in = (2j+i)*128 + p
        return np.ascontiguousarray(
            _q8(W, sc).reshape(NJ, 2, P, D).transpose(0, 2, 1, 3))

    wq_h = np.ascontiguousarray(tile_dr(Wq_e, SCQ).transpose(1, 0, 2, 3))
    wk_h = np.ascontiguousarray(tile_dr(Wk_e, SCK).transpose(1, 0, 2, 3))
    wv_h = np.ascontiguousarray(tile_dr(Wv_e, SCV).transpose(1, 0, 2, 3))
    # Wo -> [P, 2(dh)*NJ(j), 2(i), SQ] fp8 (partition-major, single DMA)
    wo_h = np.ascontiguousarray(
        _q8(Wo, SCO).reshape(NJ, 2, P, 2, SQ).transpose(3, 0, 2, 1, 4)
        .reshape(2 * NJ, P, 2, SQ).transpose(1, 0, 2, 3))
    # FFN1: f_in chunks [0, NA1) in fp8 DoubleRow pairs, rest bf16 (both *SC1)
    ka = NA1 * P
    w1a_h = np.ascontiguousarray(
        _q8(W1_e[:ka], SC1).reshape(NA1 // 2, 2, P, F)
        .transpose(2, 0, 1, 3))
    w1b_h = np.ascontiguousarray(
        (W1_e[ka:] * SC1).reshape(NF - NA1, P, NF1, P)
        .transpose(2, 1, 0, 3).reshape(NF1 // 8, 8, P, (NF - NA1) * P)
        .transpose(0, 2, 1, 3)).astype(BF)
    # FFN2: f1 chunks [0, NA2) in fp8 DoubleRow pairs, rest bf16 (both *SC2)
    kb = NA2 * P
    w2a_h = np.ascontiguousarray(
        _q8(W2[:kb], SC2).reshape(NA2 // 2, 2, P, 2, SQ)
        .transpose(3, 0, 2, 1, 4).reshape(2 * (NA2 // 2), P, 2, SQ)
        .transpose(1, 0, 2, 3))
    w2b_h = np.ascontiguousarray(
        (W2[kb:] * SC2).reshape(NF1 - NA2, P, 2, SQ)
        .transpose(2, 0, 1, 3).reshape(2 * (NF1 - NA2) // 5, 5, P, SQ)
        .transpose(0, 2, 1, 3)).astype(BF)

    flags = {
        "bq": bool(np.any(bq_e)), "bk": bool(np.any(bk_e)),
        "bv": bool(np.any(bv_e)), "bo": bool(np.any(bo)),
        "b1": bool(np.any(b1_e)), "b2": bool(np.any(b2)),
    }
    shared = {
        "ident": np.eye(P, dtype=BF),
        "wq": wq_h, "wk": wk_h, "wv": wv_h, "wo": wo_h,
        "w1a": w1a_h, "w1b": w1b_h, "w2a": w2a_h, "w2b": w2b_h,
        "bq": (bq_e * SCQ).reshape(1, D).astype(BF),
        "bk": (bk_e * SCK).reshape(1, D).astype(BF),
        "bv": (bv_e * SCV).reshape(1, D).astype(BF),
        "bo": bo.reshape(1, D).astype(np.float32),
        "b1": (b1_e * SC1).reshape(1, F).astype(BF),
        "b2": (b2 * SC2).reshape(1, D).astype(BF),
    }

    in_maps = []
    for c in range(8):
        b, hf = c // 2, c % 2
        x_rot = np.ascontiguousarray(np.roll(x[b], -SQ * hf, axis=0))
        mq = mask[b, 0, SQ * hf:SQ * (hf + 1), :]          # [512 q, 1024 k]
        mT = np.roll(mq.T, -SQ * hf, axis=0)               # [1024 k, 512 q]
        # kt-pair layout: [4, 128, 1024] with pair tiles side by side
        mT2 = np.ascontiguousarray(
            mT.reshape(NT // 2, 2, P, SQ).transpose(2, 0, 1, 3)
            .reshape(P, NT // 2, 2 * SQ)).astype(BF)
        m = dict(shared)
        m["x"] = np.ascontiguousarray(x_rot[:SQ])
        m["xk"] = np.ascontiguousarray(x_rot[SQ:]).astype(BF)
        m["maskT2"] = mT2
        in_maps.append(m)
    return in_maps, flags


def run(inputs, trace=False, **kw):
    in_maps, flags = _prep_inputs(inputs)
    nc = _get_nc(flags)
    res = run_bass_kernel_spmd(nc, in_maps, core_ids=list(range(8)),
                               trace=trace, **kw)
    out = np.empty((B, S, D), np.float32)
    for c in range(8):
        b, hf = c // 2, c % 2
        out[b, SQ * hf:SQ * (hf + 1), :] = res.results[c]["out"]
    return out, res


def kernel(**inputs) -> np.ndarray:
    out, _ = run(inputs, trace=False)
    return out
